# revision 43
# baseline (speedup 1.0000x reference)
"""HeteroGNN (2-layer GCN x 4 stacks) on 8 Trainium2 NeuronCores.

Sharding: cores {2s, 2s+1} handle stack s (jac-lnc, jac-prot, blast-lnc,
blast-prot); within a pair, destination nodes are split in halves of 25000.
Each core's 25000 dst nodes are assigned to its 196 dst-groups by a greedy
balanced (LPT) permutation of in-edge counts, so per-(group, table) slot
counts are near-identical across the 8 SPMD cores and the max-over-cores
padding shrinks from ~12.5% to ~3%.  Outputs are unpermuted on the host.

Per GCN layer, transform-first: the gather table holds dinv-prescaled
transformed rows in bf16, split into a "lo" table (rows < 32768) and a "hi"
table so row ids fit int16 for dma_gather.  Edges are bucketed by
(dst-group, lo/hi) on the host; per 2-group supertile the kernel issues
single-packet dma_gather calls of <=768 idxs on 4 rotated SWDGE queues,
builds all one-hot scatter matrices with a single wide is_equal, and
accumulates per-dst-group aggregates in PSUM via bf16 one-hot matmuls.
The dense phase-1 transform batches 8 node-chunks per wide DMA
(store via rearranged APs) and alternates HWDGE issue between the Sync and
Scalar sequencers; epilogue loads/stores are likewise batched per supertile.
Gather data buffers are memset once at startup: tail slots of partial last
blocks are never written by dma_gather, and residual NaN there would poison
the one-hot matmuls through 0*NaN even under a zero one-hot column.
The inter-layer halo exchange (pair halves of xw2) goes through the host
between two SPMD launches; the final view-combine is elementwise on host.

Measured on trn2 (8 cores): ~2.56-2.68 ms total HW exec (launch A ~1.47 ms
+ launch B ~1.11 ms; run-to-run spread +-60 us), rel-err ~5.4e-3.
Load-bearing tuning: QCHOP=768 (48+1 descs/engine; 512 is ~0.28 ms slower,
896 ~0.22 ms slower, 1008 CORRUPT), datap bufs=4 (3 costs ~0.17 ms: the
gather instruction holds the GpSimd engine while waiting out the data-tile
WAR on matmuls three supertiles back), ohp bufs=4, idxp/epp bufs=3
(4 costs ~0.2 ms).  One-hot is fp8e4 (exact 0/1; mixed fp8xbf16 matmul is
bit-identical to bf16 here) and the epilogue fuses scale+prelu into one
scalar activation when the layer biases are all zero (checked at build
time).  Known-bad variants (do not retry blindly): multi-packet big
gathers (NaN-flaky + slower), whole-run idx prefetch (+0.5 ms), bigger
SWDGE ring, fp8 gather tables (rel-err 2e-2, at the gate), GpSimd
scatter_add/ap_gather data-plane aggregation (~37 ns/idx, duplicates
dropped).  Launch B is bound by the per-supertile chain of gather-drain
completion semaphores (8 DMAHW lanes) + IS_EQ (4.9 us, input-bound: fp8
output does not speed it up) + one-hot matmuls; Q7 descriptor emission
itself is ~0.2 ns/idx when not blocked.
"""
import os
import sys

sys.path.insert(0, "/opt/trn_rl_repo")

import numpy as np
import ml_dtypes

import concourse.bass as bass
import concourse.mybir as mybir
import concourse.tile as tile
from concourse import bacc
from concourse import library_config
from concourse.bass_utils import run_bass_kernel_spmd

N = 50000
NP = 50176          # padded (392 * 128)
HALF = 25000
HP = 25088          # padded half (196 * 128)
NG = 196            # dst groups per half
NCH = 392           # node chunks for the dense transform
F_IN = 256
HID = 256
OUT = 128
P = 128
LO = 32768          # rows in the "lo" gather table (int16 range)
NHI = NP - LO       # 17408
LO_CH = LO // P     # 256 node chunks go to the lo table
G_PER = 2           # dst groups per supertile
NST = NG // G_PER   # 98 supertiles

F32 = mybir.dt.float32
BF16 = mybir.dt.bfloat16
FP8 = mybir.dt.float8e4
I16 = mybir.dt.int16
BF = ml_dtypes.bfloat16

LAST_EXEC_NS = []   # filled when KERNEL_TRACE=1


def _install_ntff_hook():
    """Register the axon NTFF profile hook (the image's antenv lacks it) and
    neuter the S3 artifact upload so tracing works offline."""
    import types, contextlib, ctypes
    import antenv  # noqa: F401
    mod = types.ModuleType("antenv.axon_hooks")
    holder = {"hook": None}
    mod.set_axon_ntff_profile_hook = lambda h: holder.__setitem__("hook", h)
    mod.get_axon_ntff_profile_hook = lambda: holder["hook"]
    sys.modules["antenv.axon_hooks"] = mod
    lib = ctypes.CDLL("/opt/axon/libaxon_pjrt.so")
    lib.axon_start_nrt_profile.argtypes = [ctypes.POINTER(ctypes.c_int64), ctypes.c_size_t]
    lib.axon_start_nrt_profile.restype = ctypes.c_int64
    lib.axon_stop_nrt_profile.argtypes = [ctypes.c_char_p]
    lib.axon_stop_nrt_profile.restype = ctypes.c_int64

    @contextlib.contextmanager
    def _hook(output_dir, device_ids):
        import jax
        jax.devices()
        if device_ids:
            ids = (ctypes.c_int64 * len(device_ids))(*device_ids)
            rc = lib.axon_start_nrt_profile(ids, len(device_ids))
        else:
            rc = lib.axon_start_nrt_profile(None, 0)
        if rc != 0:
            raise RuntimeError(f"axon_start_nrt_profile rc={rc}")
        try:
            yield
        finally:
            lib.axon_stop_nrt_profile(str(output_dir).encode())

    mod.set_axon_ntff_profile_hook(_hook)
    from concourse import bass_utils
    bass_utils.upload_artifacts = lambda tmpdir: str(tmpdir)


def _cdiv(a, b):
    return -(-a // b)


def _balance_pos(w):
    """Assign HALF dst nodes to NG groups of <=P lanes each, balancing total
    in-edge weight per group (greedy LPT).  Near-equal group loads on every
    core shrink the SPMD padding (slot counts are max-over-cores)."""
    import heapq
    order = np.argsort(-w, kind="stable")
    fill = np.zeros(NG, dtype=np.int64)
    pos = np.empty(w.shape[0], dtype=np.int64)
    hp = [(0.0, g) for g in range(NG)]
    heapq.heapify(hp)
    for d in order:
        while True:
            l, g = heapq.heappop(hp)
            if fill[g] < P:
                break
        pos[d] = g * P + fill[g]
        fill[g] += 1
        if fill[g] < P:
            heapq.heappush(hp, (l + float(w[d]), g))
    return pos


QCHOP = 768  # idxs per dma_gather call; 1008 (=63+1 descs/engine) corrupts data


def _big_gather(nc, data3, tbl, idx_t, col0, total, elem, blk0, qctr):
    """Single-packet dma_gather calls of <=QCHOP idxs covering `total` slots.
    Multi-packet big calls measured slower and NaN-flaky under profiling;
    chopped single-packet on rotated SWDGE queues is the best known config,
    with 768 (48+1 descs/engine) the measured sweet spot."""
    off = 0
    while off < total:
        n = min(QCHOP, total - off)
        b0 = blk0 + off // P
        nc.gpsimd.dma_gather(
            data3[:, b0:b0 + _cdiv(n, P), :], tbl[:],
            idx_t[:, col0 + off // 16:col0 + off // 16 + _cdiv(n, 16)],
            n, n, elem, single_packet=True,
            queue_num=(qctr[0] // 2) % 4)
        qctr[0] += 1
        off += n


def _build_layout(cnt_lo, cnt_hi):
    """Static supertile layout shared by all 8 cores.

    cnt_lo/cnt_hi: [8, NG] per-core edge counts per (dst-group, table-kind).
    Returns per-supertile dicts with slot offsets, matmul lists, dmod/idx
    column bases.
    """
    SL = cnt_lo.max(axis=0).astype(np.int64)
    SH = cnt_hi.max(axis=0).astype(np.int64)
    sts = []
    m_base = 0
    ci_base = 0
    for t in range(NST):
        gs = list(range(G_PER * t, G_PER * (t + 1)))
        sL = [int(SL[g]) for g in gs]
        sH = [int(SH[g]) for g in gs]
        sumSL, sumSH = sum(sL), sum(sH)
        nbL, nbH = _cdiv(sumSL, P), _cdiv(sumSH, P)
        offL, offH = {}, {}
        o = 0
        for g, s in zip(gs, sL):
            offL[g] = o
            o += s
        o = 0
        for g, s in zip(gs, sH):
            offH[g] = o
            o += s
        mms = []  # (j_tile, g, kind, j_call)
        for kind in ("lo", "hi"):
            sumS = sumSL if kind == "lo" else sumSH
            nb = nbL if kind == "lo" else nbH
            offs = offL if kind == "lo" else offH
            S = sL if kind == "lo" else sH
            for j in range(nb):
                blk_a, blk_b = j * P, min((j + 1) * P, sumS)
                for gi, g in enumerate(gs):
                    a = offs[g]
                    b = a + S[gi]
                    if a < blk_b and b > blk_a and S[gi] > 0:
                        jt = j if kind == "lo" else nbL + j
                        mms.append((jt, g, kind, j))
        first, last = {}, {}
        for mi, (jt, g, kind, j) in enumerate(mms):
            first.setdefault(g, mi)
            last[g] = mi
        CL, CH = _cdiv(sumSL, 16), _cdiv(sumSH, 16)
        sts.append(dict(gs=gs, sL=sL, sH=sH, sumSL=sumSL, sumSH=sumSH,
                        nbL=nbL, nbH=nbH, offL=offL, offH=offH,
                        mms=mms, first=first, last=last,
                        m0=m_base, ci0=ci_base, CL=CL, CH=CH))
        m_base += len(mms)
        ci_base += CL + CH
    return SL, SH, sts, m_base, ci_base


def _core_tables(src_r, dst_l, sts, nm_tot, ic_tot):
    """Per-core idx (int16, [128, ic_tot]) and dmod (f32 -> bf16, [128, nm_tot])."""
    g = dst_l // P
    d = dst_l % P
    kindi = (src_r >= LO).astype(np.int64)
    order = np.lexsort((src_r, kindi, g))
    sg, sk, ss, sd = g[order], kindi[order], src_r[order], d[order]
    cnt = np.bincount(sg * 2 + sk, minlength=NG * 2)
    starts = np.concatenate([[0], np.cumsum(cnt)[:-1]]).reshape(NG, 2)
    cnt = cnt.reshape(NG, 2)

    idx16 = np.zeros((16, ic_tot), dtype=np.int16)
    dmod = np.full((P, nm_tot), 255.0, dtype=np.float32)
    for st in sts:
        dva_k = {}
        for kind in ("lo", "hi"):
            k = 0 if kind == "lo" else 1
            sumS = st["sumSL"] if kind == "lo" else st["sumSH"]
            nb = st["nbL"] if kind == "lo" else st["nbH"]
            offs = st["offL"] if kind == "lo" else st["offH"]
            ci = st["ci0"] if kind == "lo" else st["ci0"] + st["CL"]
            val = np.zeros(nb * P, dtype=np.int64)
            dva = np.full(nb * P, 255, dtype=np.int64)
            for g_ in st["gs"]:
                c = int(cnt[g_, k])
                s0 = int(starts[g_, k])
                a = offs[g_]
                if c:
                    val[a:a + c] = ss[s0:s0 + c] - (0 if kind == "lo" else LO)
                    dva[a:a + c] = sd[s0:s0 + c]
            if sumS:
                s_arr = np.arange(sumS)
                idx16[s_arr % 16, ci + s_arr // 16] = val[:sumS].astype(np.int16)
            dva_k[kind] = dva
        for mi, (jt, g_, kind, j) in enumerate(st["mms"]):
            dva = dva_k[kind]
            offs = st["offL"] if kind == "lo" else st["offH"]
            S = st["sL"] if kind == "lo" else st["sH"]
            gi = st["gs"].index(g_)
            a = offs[g_]
            b = a + S[gi]
            sl = j * P + np.arange(P)
            dmod[:, st["m0"] + mi] = np.where((sl >= a) & (sl < b), dva[sl], 255)
    return np.tile(idx16, (8, 1)), dmod


def _build_a(sts, nm_tot, ic_tot, nmm, nbmax, icmax, zb1=False):
    nc = bacc.Bacc("TRN2", target_bir_lowering=False, debug=False, num_devices=8,
                   num_swdge_queues=4)
    xT = nc.dram_tensor("xT", [F_IN, NP], BF16, kind="ExternalInput")
    W1 = nc.dram_tensor("W1", [F_IN, HID + OUT], BF16, kind="ExternalInput")
    W2 = nc.dram_tensor("W2", [HID, OUT], BF16, kind="ExternalInput")
    b1t_d = nc.dram_tensor("b1t", [P, HID], F32, kind="ExternalInput")
    dnod_d = nc.dram_tensor("dnod", [P, NCH], F32, kind="ExternalInput")
    ddst_d = nc.dram_tensor("ddst", [P, NG], F32, kind="ExternalInput")
    iota_d = nc.dram_tensor("iota3", [P, nmm, P], BF16, kind="ExternalInput")
    ident_d = nc.dram_tensor("ident", [P, P], BF16, kind="ExternalInput")
    dmod_d = nc.dram_tensor("dmod", [P, nm_tot], BF16, kind="ExternalInput")
    idx_d = nc.dram_tensor("idx", [P, ic_tot], I16, kind="ExternalInput")
    xw1_lo = nc.dram_tensor("xw1_lo", [LO, HID], BF16)
    xw1_hi = nc.dram_tensor("xw1_hi", [NHI, HID], BF16)
    xw2_o = nc.dram_tensor("xw2_own", [HP, OUT], BF16, kind="ExternalOutput")
    res_o = nc.dram_tensor("res_own", [HP, OUT], BF16, kind="ExternalOutput")

    ACT = mybir.ActivationFunctionType

    with tile.TileContext(nc) as tc:
        nc.gpsimd.load_library(library_config.mlp)
        with tc.tile_pool(name="const", bufs=1) as cp:
            w1a = cp.tile([P, HID + OUT], BF16); nc.sync.dma_start(out=w1a[:], in_=W1[0:P, :])
            w1b = cp.tile([P, HID + OUT], BF16); nc.sync.dma_start(out=w1b[:], in_=W1[P:2 * P, :])
            w2a = cp.tile([P, OUT], BF16); nc.sync.dma_start(out=w2a[:], in_=W2[0:P, :])
            w2b = cp.tile([P, OUT], BF16); nc.sync.dma_start(out=w2b[:], in_=W2[P:2 * P, :])
            b1t = cp.tile([P, HID], F32); nc.sync.dma_start(out=b1t[:], in_=b1t_d[:])
            dn_t = cp.tile([P, NCH], F32); nc.sync.dma_start(out=dn_t[:], in_=dnod_d[:])
            dd_t = cp.tile([P, NG], F32); nc.sync.dma_start(out=dd_t[:], in_=ddst_d[:])
            iota = cp.tile([P, nmm, P], BF16); nc.sync.dma_start(out=iota[:], in_=iota_d[:])
            ident = cp.tile([P, P], BF16); nc.sync.dma_start(out=ident[:], in_=ident_d[:])
            dmod_t = cp.tile([P, nm_tot], BF16); nc.sync.dma_start(out=dmod_t[:], in_=dmod_d[:])

            # step 1: xw1[n] = dinv[n] * (x[n] @ W1), bf16 tables; residual for own half
            # 8 chunks per iteration; one wide store per table / per res batch
            CB = 8
            with (
                tc.tile_pool(name="xt", bufs=4) as xtp,
                tc.tile_pool(name="mm1", bufs=4, space="PSUM") as mm1p,
                tc.tile_pool(name="sb1", bufs=3) as sb1p,
            ):
              for cc in range(NCH // CB):
                  xa = xtp.tile([P, CB * P], BF16, tag="xt")
                  nc.sync.dma_start(out=xa[:], in_=xT[0:P, cc * CB * P:(cc + 1) * CB * P])
                  xb = xtp.tile([P, CB * P], BF16, tag="xt")
                  nc.sync.dma_start(out=xb[:], in_=xT[P:2 * P, cc * CB * P:(cc + 1) * CB * P])
                  tw = sb1p.tile([P, CB, HID], BF16, tag="tw")
                  nres = min(max(NG - cc * CB, 0), CB)
                  rw = None
                  if nres:
                      rw = sb1p.tile([P, CB, OUT], BF16, tag="rw", name="rw")
                  for j in range(CB):
                      c = cc * CB + j
                      wid = HID + OUT if c < NG else HID
                      ps = mm1p.tile([P, HID + OUT], F32, tag="mm1")
                      nc.tensor.matmul(out=ps[:, 0:wid], lhsT=xa[:, j * P:(j + 1) * P],
                                       rhs=w1a[:, 0:wid], start=True, stop=False)
                      nc.tensor.matmul(out=ps[:, 0:wid], lhsT=xb[:, j * P:(j + 1) * P],
                                       rhs=w1b[:, 0:wid], start=False, stop=True)
                      if c % 2 == 0:
                          nc.scalar.activation(out=tw[:, j, :], in_=ps[:, 0:HID],
                                               func=ACT.Copy, scale=dn_t[:, c:c + 1])
                      else:
                          nc.vector.tensor_tensor(
                              out=tw[:, j, :], in0=dn_t[:, c:c + 1].to_broadcast([P, HID]),
                              in1=ps[:, 0:HID], op=mybir.AluOpType.mult)
                      if c < NG:
                          nc.vector.tensor_copy(out=rw[:, j, :], in_=ps[:, HID:HID + OUT])
                  if nres:
                      nc.scalar.dma_start(
                          out=res_o[cc * CB * P:(cc * CB + nres) * P, :]
                              .rearrange("(j p) f -> p j f", p=P),
                          in_=rw[:, 0:nres, :])
                  if cc < LO_CH // CB:
                      nc.sync.dma_start(
                          out=xw1_lo[cc * CB * P:(cc + 1) * CB * P, :]
                              .rearrange("(j p) f -> p j f", p=P),
                          in_=tw[:])
                  else:
                      cq = cc - LO_CH // CB
                      nc.sync.dma_start(
                          out=xw1_hi[cq * CB * P:(cq + 1) * CB * P, :]
                              .rearrange("(j p) f -> p j f", p=P),
                          in_=tw[:])

            tc.strict_bb_all_engine_barrier()

            # step 2: per-supertile gather + scatter-matmul + epilogue
            qctr = [0]
            with (
                tc.tile_pool(name="idx", bufs=3) as idxp,
                tc.tile_pool(name="data", bufs=4) as datap,
                tc.tile_pool(name="oh", bufs=4) as ohp,
                tc.tile_pool(name="agg", bufs=2 * G_PER, space="PSUM") as aggp,
                tc.tile_pool(name="tp", bufs=2, space="PSUM") as tpp,
                tc.tile_pool(name="mm2", bufs=2, space="PSUM") as mm2p,
                tc.tile_pool(name="ep", bufs=3) as epp,
            ):
              # clear the gather buffers: tail slots of partial last blocks are
              # never written by dma_gather; residual NaN there would poison the
              # one-hot matmuls (0*NaN=NaN) even under a zero one-hot column.
              for _ in range(4):
                  z = datap.tile([P, nbmax, HID], BF16, tag="data", name="zi")
                  nc.vector.memset(z[:], 0.0)
              for st in sts:
                  nbL, nbH = st["nbL"], st["nbH"]
                  nb = nbL + nbH
                  ict = st["CL"] + st["CH"]
                  nmt = len(st["mms"])
                  g0 = st["gs"][0]
                  idx_t = idxp.tile([P, icmax], I16, tag="idx")
                  nc.sync.dma_start(out=idx_t[:, 0:ict],
                                    in_=idx_d[:, st["ci0"]:st["ci0"] + ict])
                  data = datap.tile([P, nbmax, HID], BF16, tag="data")
                  if st["sumSL"]:
                      _big_gather(nc, data, xw1_lo, idx_t, 0,
                                  st["sumSL"], HID, 0, qctr)
                  if st["sumSH"]:
                      _big_gather(nc, data, xw1_hi, idx_t, st["CL"],
                                  st["sumSH"], HID, nbL, qctr)
                  slf2 = epp.tile([P, G_PER, HID], BF16, tag="slf")
                  nc.scalar.dma_start(
                      out=slf2[:],
                      in_=xw1_lo[g0 * P:(g0 + G_PER) * P, :]
                          .rearrange("(j p) f -> p j f", p=P))
                  xw2w = epp.tile([P, G_PER, OUT], BF16, tag="xw2w")
                  oh = ohp.tile([P, nmm, P], FP8, tag="oh")
                  nc.vector.tensor_tensor(
                      out=oh[:, 0:nmt, :],
                      in0=dmod_t[:, st["m0"]:st["m0"] + nmt].to_broadcast([P, nmt, P]),
                      in1=iota[:, 0:nmt, :], op=mybir.AluOpType.is_equal)
                  aggs = {}
                  for g in st["gs"]:
                      aggs[g] = aggp.tile([P, HID], F32, tag="agg", name=f"agg{g}")
                  for mi, (jt, g, kind, j) in enumerate(st["mms"]):
                      nc.tensor.matmul(out=aggs[g][:], lhsT=oh[:, mi, :],
                                       rhs=data[:, jt, :],
                                       start=(st["first"][g] == mi),
                                       stop=(st["last"][g] == mi))
                      if st["last"][g] != mi:
                          continue
                      # epilogue for group g
                      gi = st["gs"].index(g)
                      s = epp.tile([P, HID], F32, tag="s")
                      nc.vector.tensor_add(out=s[:], in0=aggs[g][:], in1=slf2[:, gi, :])
                      h = epp.tile([P, HID], BF16, tag="h")
                      if zb1:
                          # bias is all-zero: h = prelu(s * dinv_d) in one op
                          nc.scalar.activation(out=h[:], in_=s[:], func=ACT.Prelu,
                                               scale=dd_t[:, g:g + 1], alpha=0.2)
                      else:
                          s2 = epp.tile([P, HID], F32, tag="s2")
                          nc.scalar.activation(out=s2[:], in_=s[:], func=ACT.Copy,
                                               scale=dd_t[:, g:g + 1])
                          s3 = epp.tile([P, HID], F32, tag="s3")
                          nc.vector.tensor_add(out=s3[:], in0=s2[:], in1=b1t[:])
                          nc.scalar.activation(out=h[:], in_=s3[:], func=ACT.Prelu,
                                               alpha=0.2)
                      pt = tpp.tile([P, P], BF16, tag="pt")
                      nc.tensor.transpose(out=pt[:], in_=h[:, 0:P], identity=ident[:])
                      hta = epp.tile([P, P], BF16, tag="hta")
                      nc.vector.tensor_copy(out=hta[:], in_=pt[:])
                      pt2 = tpp.tile([P, P], BF16, tag="pt")
                      nc.tensor.transpose(out=pt2[:], in_=h[:, P:2 * P], identity=ident[:])
                      htb = epp.tile([P, P], BF16, tag="htb")
                      nc.vector.tensor_copy(out=htb[:], in_=pt2[:])
                      ps2 = mm2p.tile([P, OUT], F32, tag="mm2")
                      nc.tensor.matmul(out=ps2[:], lhsT=hta[:], rhs=w2a[:],
                                       start=True, stop=False)
                      nc.tensor.matmul(out=ps2[:], lhsT=htb[:], rhs=w2b[:],
                                       start=False, stop=True)
                      nc.scalar.activation(out=xw2w[:, gi, :], in_=ps2[:], func=ACT.Copy,
                                           scale=dd_t[:, g:g + 1])
                  nc.scalar.dma_start(
                      out=xw2_o[g0 * P:(g0 + G_PER) * P, :]
                          .rearrange("(j p) f -> p j f", p=P),
                      in_=xw2w[:])
    nc.compile()
    return nc


def _build_b(sts, nm_tot, ic_tot, nmm, nbmax, icmax, zb2=False):
    nc = bacc.Bacc("TRN2", target_bir_lowering=False, debug=False, num_devices=8,
                   num_swdge_queues=4)
    xw2_lo = nc.dram_tensor("xw2_lo", [LO, OUT], BF16, kind="ExternalInput")
    xw2_hi = nc.dram_tensor("xw2_hi", [NHI, OUT], BF16, kind="ExternalInput")
    b2t_d = nc.dram_tensor("b2t", [P, OUT], F32, kind="ExternalInput")
    ddst_d = nc.dram_tensor("ddst", [P, NG], F32, kind="ExternalInput")
    iota_d = nc.dram_tensor("iota3", [P, nmm, P], BF16, kind="ExternalInput")
    dmod_d = nc.dram_tensor("dmod", [P, nm_tot], BF16, kind="ExternalInput")
    idx_d = nc.dram_tensor("idx", [P, ic_tot], I16, kind="ExternalInput")
    out_o = nc.dram_tensor("out_own", [HP, OUT], BF16, kind="ExternalOutput")

    ACT = mybir.ActivationFunctionType

    with tile.TileContext(nc) as tc:
        nc.gpsimd.load_library(library_config.mlp)
        qctr = [0]
        with (
            tc.tile_pool(name="const", bufs=1) as cp,
            tc.tile_pool(name="idx", bufs=3) as idxp,
            tc.tile_pool(name="data", bufs=4) as datap,
            tc.tile_pool(name="oh", bufs=4) as ohp,
            tc.tile_pool(name="agg", bufs=6, space="PSUM") as aggp,
            tc.tile_pool(name="ep", bufs=3) as epp,
        ):
            b2t = cp.tile([P, OUT], F32); nc.sync.dma_start(out=b2t[:], in_=b2t_d[:])
            dd_t = cp.tile([P, NG], F32); nc.sync.dma_start(out=dd_t[:], in_=ddst_d[:])
            iota = cp.tile([P, nmm, P], BF16); nc.sync.dma_start(out=iota[:], in_=iota_d[:])
            dmod_t = cp.tile([P, nm_tot], BF16); nc.sync.dma_start(out=dmod_t[:], in_=dmod_d[:])

            # see _build_a: clear gather buffers against 0*NaN poisoning
            for _ in range(4):
                z = datap.tile([P, nbmax, OUT], BF16, tag="data", name="zi")
                nc.vector.memset(z[:], 0.0)
            for st in sts:
                nbL, nbH = st["nbL"], st["nbH"]
                nb = nbL + nbH
                ict = st["CL"] + st["CH"]
                nmt = len(st["mms"])
                g0 = st["gs"][0]
                idx_t = idxp.tile([P, icmax], I16, tag="idx")
                nc.sync.dma_start(out=idx_t[:, 0:ict],
                                  in_=idx_d[:, st["ci0"]:st["ci0"] + ict])
                data = datap.tile([P, nbmax, OUT], BF16, tag="data")
                if st["sumSL"]:
                    _big_gather(nc, data, xw2_lo, idx_t, 0,
                                st["sumSL"], OUT, 0, qctr)
                if st["sumSH"]:
                    _big_gather(nc, data, xw2_hi, idx_t, st["CL"],
                                st["sumSH"], OUT, nbL, qctr)
                slf2 = epp.tile([P, G_PER, OUT], BF16, tag="slf")
                nc.scalar.dma_start(
                    out=slf2[:],
                    in_=xw2_lo[g0 * P:(g0 + G_PER) * P, :]
                        .rearrange("(j p) f -> p j f", p=P))
                ow = epp.tile([P, G_PER, OUT], BF16, tag="ow")
                oh = ohp.tile([P, nmm, P], FP8, tag="oh")
                nc.vector.tensor_tensor(
                    out=oh[:, 0:nmt, :],
                    in0=dmod_t[:, st["m0"]:st["m0"] + nmt].to_broadcast([P, nmt, P]),
                    in1=iota[:, 0:nmt, :], op=mybir.AluOpType.is_equal)
                aggs = {}
                for g in st["gs"]:
                    aggs[g] = aggp.tile([P, OUT], F32, tag="agg", name=f"agg{g}")
                for mi, (jt, g, kind, j) in enumerate(st["mms"]):
                    nc.tensor.matmul(out=aggs[g][:], lhsT=oh[:, mi, :],
                                     rhs=data[:, jt, :],
                                     start=(st["first"][g] == mi),
                                     stop=(st["last"][g] == mi))
                    if st["last"][g] != mi:
                        continue
                    gi = st["gs"].index(g)
                    s = epp.tile([P, OUT], F32, tag="s")
                    nc.vector.tensor_add(out=s[:], in0=aggs[g][:], in1=slf2[:, gi, :])
                    if zb2:
                        nc.scalar.activation(out=ow[:, gi, :], in_=s[:], func=ACT.Prelu,
                                             scale=dd_t[:, g:g + 1], alpha=0.2)
                    else:
                        s2 = epp.tile([P, OUT], F32, tag="s2")
                        nc.scalar.activation(out=s2[:], in_=s[:], func=ACT.Copy,
                                             scale=dd_t[:, g:g + 1])
                        s3 = epp.tile([P, OUT], F32, tag="s3")
                        nc.vector.tensor_add(out=s3[:], in0=s2[:], in1=b2t[:])
                        nc.scalar.activation(out=ow[:, gi, :], in_=s3[:], func=ACT.Prelu,
                                             alpha=0.2)
                nc.scalar.dma_start(
                    out=out_o[g0 * P:(g0 + G_PER) * P, :]
                        .rearrange("(j p) f -> p j f", p=P),
                    in_=ow[:])
    nc.compile()
    return nc


def kernel(**inputs):
    trace = os.environ.get("KERNEL_TRACE", "0") == "1"
    if trace:
        try:
            _install_ntff_hook()
        except Exception:
            trace = False

    f32 = np.float32
    stacks = [
        ("x_lnc_jac", "edge_jac_lnc", "W_j1_lnc", "b_j1_lnc", "W_j2_lnc", "b_j2_lnc", "W_res_lnc"),
        ("x_prot_jac", "edge_jac_prot", "W_j1_prot", "b_j1_prot", "W_j2_prot", "b_j2_prot", "W_res_prot"),
        ("x_lnc_blast", "edge_blast_lnc", "W_b1_lnc", "b_b1_lnc", "W_b2_lnc", "b_b2_lnc", None),
        ("x_prot_blast", "edge_blast_prot", "W_b1_prot", "b_b1_prot", "W_b2_prot", "b_b2_prot", None),
    ]

    # per-core edge preprocessing.  Pass 1: per-core balanced dst->row
    # permutation (equalizes per-group edge counts so the SPMD max-over-cores
    # slot padding nearly vanishes).  Pass 2: relabel edges through the
    # row maps (partner srcs use the partner core's permutation).
    raw = []
    pos_owns = []
    for c in range(8):
        sname = stacks[c // 2]
        h = c % 2
        e = np.asarray(inputs[sname[1]])
        src, dst = e[0].astype(np.int64), e[1].astype(np.int64)
        deg = (np.bincount(dst, minlength=N) + 1.0).astype(f32)
        dinv = (1.0 / np.sqrt(deg)).astype(f32)
        sel = (dst >= h * HALF) & (dst < (h + 1) * HALF)
        srcs, dsts = src[sel], dst[sel] - h * HALF
        w = np.bincount(dsts, minlength=HALF)
        pos_owns.append(_balance_pos(w))
        raw.append((srcs, dsts, dinv))

    pre = []
    cnt_lo = np.zeros((8, NG), dtype=np.int64)
    cnt_hi = np.zeros((8, NG), dtype=np.int64)
    for c in range(8):
        h = c % 2
        srcs, dsts, dinv = raw[c]
        pos_own, pos_par = pos_owns[c], pos_owns[c ^ 1]
        own = (srcs >= h * HALF) & (srcs < (h + 1) * HALF)
        src_r = np.where(own, pos_own[np.clip(srcs - h * HALF, 0, HALF - 1)],
                         HP + pos_par[np.clip(srcs - (1 - h) * HALF, 0, HALF - 1)])
        dst_l = pos_own[dsts]
        gi = dst_l // P
        ki = (src_r >= LO).astype(np.int64)
        cnt_lo[c] = np.bincount(gi[ki == 0], minlength=NG)
        cnt_hi[c] = np.bincount(gi[ki == 1], minlength=NG)
        pre.append((src_r, dst_l, dinv))

    SL, SH, sts, nm_tot, ic_tot = _build_layout(cnt_lo, cnt_hi)
    if os.environ.get("KERNEL_DEBUG", "0") == "1":
        slots = int(SL.sum() + SH.sum())
        print(f"dbg slots={slots} edges/core~{cnt_lo.sum(1).mean()+cnt_hi.sum(1).mean():.0f} "
              f"pad={slots / (cnt_lo.sum(1).mean()+cnt_hi.sum(1).mean()) - 1:.4f}",
              flush=True)
    nmm = max(len(st["mms"]) for st in sts)
    nbmax = max(st["nbL"] + st["nbH"] for st in sts)
    icmax = max(st["CL"] + st["CH"] for st in sts)

    iota3 = np.broadcast_to(np.arange(P, dtype=f32), (P, nmm, P)).astype(BF).copy()
    ident = np.eye(P, dtype=f32).astype(BF)

    in_a, in_b_partial = [], []
    for c in range(8):
        sname = stacks[c // 2]
        h = c % 2
        src_r, dst_l, dinv = pre[c]
        idx_full, dmod = _core_tables(src_r, dst_l, sts, nm_tot, ic_tot)
        x = np.asarray(inputs[sname[0]], dtype=f32)
        nodeatrow = np.full(NP, -1, dtype=np.int64)
        nodeatrow[pos_owns[c]] = h * HALF + np.arange(HALF)
        nodeatrow[HP + pos_owns[c ^ 1]] = (1 - h) * HALF + np.arange(HALF)
        valid = nodeatrow >= 0
        xT = np.zeros((F_IN, NP), dtype=f32)
        xT[:, valid] = x[nodeatrow[valid]].T
        dpad = np.ones(NP, dtype=f32)
        dpad[valid] = dinv[nodeatrow[valid]]
        dnod = dpad[np.arange(NP, dtype=np.int64).reshape(NCH, P).T].astype(f32)
        ddst = dpad[np.arange(HP, dtype=np.int64).reshape(NG, P).T].astype(f32)
        W1 = np.asarray(inputs[sname[2]], dtype=f32)
        b1 = np.asarray(inputs[sname[3]], dtype=f32)
        W2 = np.asarray(inputs[sname[4]], dtype=f32)
        b2 = np.asarray(inputs[sname[5]], dtype=f32)
        Wr = np.asarray(inputs[sname[6]], dtype=f32) if sname[6] else np.zeros((F_IN, OUT), dtype=f32)
        dmod_bf = dmod.astype(BF)
        xT_bf = xT.astype(BF)
        in_a.append({
            "xT": xT_bf, "W1": np.concatenate([W1, Wr], axis=1).astype(BF),
            "W2": W2.astype(BF),
            "b1t": np.broadcast_to(b1, (P, HID)).copy(),
            "dnod": np.ascontiguousarray(dnod), "ddst": np.ascontiguousarray(ddst),
            "iota3": iota3, "ident": ident,
            "dmod": dmod_bf, "idx": idx_full,
        })
        in_b_partial.append({
            "b2t": np.broadcast_to(b2, (P, OUT)).copy(),
            "ddst": np.ascontiguousarray(ddst),
            "iota3": iota3, "dmod": dmod_bf, "idx": idx_full,
        })

    import tempfile
    zb1 = all(not np.any(np.asarray(inputs[stacks[s][3]])) for s in range(4))
    zb2 = all(not np.any(np.asarray(inputs[stacks[s][5]])) for s in range(4))
    nc_a = _build_a(sts, nm_tot, ic_tot, nmm, nbmax, icmax, zb1=zb1)
    res_a = run_bass_kernel_spmd(nc_a, in_a, list(range(8)), trace=trace,
                                 tmpdir=tempfile.mkdtemp(prefix="gnn_a_") if trace else None)
    LAST_EXEC_NS.clear()
    if trace and res_a.exec_time_ns:
        LAST_EXEC_NS.append(res_a.exec_time_ns)

    if os.environ.get("KERNEL_DEBUG", "0") == "1":
        for c in range(8):
            xo = np.asarray(res_a.results[c]["xw2_own"]).astype(np.float32)
            ro = np.asarray(res_a.results[c]["res_own"]).astype(np.float32)
            print(f"dbg core {c}: xw2 nan%={np.isnan(xo).mean():.4f} "
                  f"max={np.nanmax(np.abs(xo)):.3f}  res nan%={np.isnan(ro).mean():.4f} "
                  f"max={np.nanmax(np.abs(ro)):.3f}", flush=True)
        xo = np.asarray(res_a.results[0]["xw2_own"]).astype(np.float32)
        nanrow = np.isnan(xo).any(axis=1)
        print("dbg nan rows core0:", np.where(nanrow)[0][:20], nanrow.sum(), flush=True)

    # host halo exchange: assemble full xw2 per pair (bf16 concat only;
    # both halves are already in their core's permuted row order)
    in_b = []
    for c in range(8):
        partner = c ^ 1
        xw2f = np.zeros((NP, OUT), dtype=BF)
        xw2f[:HP] = np.asarray(res_a.results[c]["xw2_own"])
        xw2f[HP:] = np.asarray(res_a.results[partner]["xw2_own"])
        in_b.append({"xw2_lo": xw2f[:LO].copy(), "xw2_hi": xw2f[LO:].copy(),
                     **in_b_partial[c]})

    nc_b = _build_b(sts, nm_tot, ic_tot, nmm, nbmax, icmax, zb2=zb2)
    res_b = run_bass_kernel_spmd(nc_b, in_b, list(range(8)), trace=trace,
                                 tmpdir=tempfile.mkdtemp(prefix="gnn_b_") if trace else None)
    if trace and res_b.exec_time_ns:
        LAST_EXEC_NS.append(res_b.exec_time_ns)

    def full_out(pair):
        return np.concatenate([
            np.asarray(res_b.results[2 * pair]["out_own"])[pos_owns[2 * pair]],
            np.asarray(res_b.results[2 * pair + 1]["out_own"])[pos_owns[2 * pair + 1]],
        ]).astype(f32)

    jl, jp, bl, bp = full_out(0), full_out(1), full_out(2), full_out(3)
    res_l = np.concatenate(
        [np.asarray(res_a.results[0]["res_own"])[pos_owns[0]],
         np.asarray(res_a.results[1]["res_own"])[pos_owns[1]]]).astype(f32) \
        + np.asarray(inputs["b_res_lnc"], dtype=f32)
    res_p = np.concatenate(
        [np.asarray(res_a.results[2]["res_own"])[pos_owns[2]],
         np.asarray(res_a.results[3]["res_own"])[pos_owns[3]]]).astype(f32) \
        + np.asarray(inputs["b_res_prot"], dtype=f32)
    comb_l = (jl + bl) * 0.5 + res_l
    comb_p = (jp + bp) * 0.5 + res_p
    return (comb_l, comb_p, jl, jp, bl, bp)



# revision 44
# speedup vs baseline: 1.3127x; 1.3127x over previous
"""HeteroGNN (2-layer GCN x 4 stacks) on 8 Trainium2 NeuronCores.

Sharding: cores {2s, 2s+1} handle stack s (jac-lnc, jac-prot, blast-lnc,
blast-prot); within a pair, destination nodes are split in halves of 25000.
Each core's 25000 dst nodes are assigned to its 196 dst-groups by a greedy
balanced (LPT) permutation of in-edge counts, so per-(group, table) slot
counts are near-identical across the 8 SPMD cores and the max-over-cores
padding shrinks from ~12.5% to ~3%.  Outputs are unpermuted on the host.

Per GCN layer, transform-first: the gather table holds dinv-prescaled
transformed rows in bf16, split into a "lo" table (rows < 32768) and a "hi"
table so row ids fit int16 for dma_gather.  Edges are bucketed by
(dst-group, lo/hi) on the host; per 2-group supertile the kernel issues
single-packet dma_gather calls of <=768 idxs on 4 rotated SWDGE queues,
builds all one-hot scatter matrices with a single wide is_equal, and
accumulates per-dst-group aggregates in PSUM via bf16 one-hot matmuls.
The dense phase-1 transform batches 8 node-chunks per wide DMA
(store via rearranged APs) and alternates HWDGE issue between the Sync and
Scalar sequencers; epilogue loads/stores are likewise batched per supertile.
Gather data buffers are memset once at startup: tail slots of partial last
blocks are never written by dma_gather, and residual NaN there would poison
the one-hot matmuls through 0*NaN even under a zero one-hot column.
The inter-layer halo exchange (pair halves of xw2) goes through the host
between two SPMD launches; the final view-combine is elementwise on host.

Measured on trn2 (8 cores): ~2.56-2.68 ms total HW exec (launch A ~1.47 ms
+ launch B ~1.11 ms; run-to-run spread +-60 us), rel-err ~5.4e-3.
Load-bearing tuning: QCHOP=768 (48+1 descs/engine; 512 is ~0.28 ms slower,
896 ~0.22 ms slower, 1008 CORRUPT), datap bufs=4 (3 costs ~0.17 ms: the
gather instruction holds the GpSimd engine while waiting out the data-tile
WAR on matmuls three supertiles back), ohp bufs=4, idxp/epp bufs=3
(4 costs ~0.2 ms).  One-hot is fp8e4 (exact 0/1; mixed fp8xbf16 matmul is
bit-identical to bf16 here) and the epilogue fuses scale+prelu into one
scalar activation when the layer biases are all zero (checked at build
time).  Known-bad variants (do not retry blindly): multi-packet big
gathers (NaN-flaky + slower), whole-run idx prefetch (+0.5 ms), bigger
SWDGE ring, fp8 gather tables (rel-err 2e-2, at the gate), GpSimd
scatter_add/ap_gather data-plane aggregation (~37 ns/idx, duplicates
dropped).  Launch B is bound by the per-supertile chain of gather-drain
completion semaphores (8 DMAHW lanes) + IS_EQ (4.9 us, input-bound: fp8
output does not speed it up) + one-hot matmuls; Q7 descriptor emission
itself is ~0.2 ns/idx when not blocked.
"""
import os
import sys

sys.path.insert(0, "/opt/trn_rl_repo")

import numpy as np
import ml_dtypes

import concourse.bass as bass
import concourse.mybir as mybir
import concourse.tile as tile
from concourse import bacc
from concourse import library_config
from concourse.bass_utils import run_bass_kernel_spmd

N = 50000
NP = 50176          # padded (392 * 128)
HALF = 25000
HP = 25088          # padded half (196 * 128)
NG = 196            # dst groups per half
NCH = 392           # node chunks for the dense transform
F_IN = 256
HID = 256
OUT = 128
P = 128
LO = 32768          # rows in the "lo" gather table (int16 range)
NHI = NP - LO       # 17408
LO_CH = LO // P     # 256 node chunks go to the lo table
G_PER = 2           # dst groups per supertile
NST = NG // G_PER   # 98 supertiles

F32 = mybir.dt.float32
BF16 = mybir.dt.bfloat16
FP8 = mybir.dt.float8e4
I16 = mybir.dt.int16
BF = ml_dtypes.bfloat16

LAST_EXEC_NS = []   # filled when KERNEL_TRACE=1


def _install_ntff_hook():
    """Register the axon NTFF profile hook (the image's antenv lacks it) and
    neuter the S3 artifact upload so tracing works offline."""
    import types, contextlib, ctypes
    import antenv  # noqa: F401
    mod = types.ModuleType("antenv.axon_hooks")
    holder = {"hook": None}
    mod.set_axon_ntff_profile_hook = lambda h: holder.__setitem__("hook", h)
    mod.get_axon_ntff_profile_hook = lambda: holder["hook"]
    sys.modules["antenv.axon_hooks"] = mod
    lib = ctypes.CDLL("/opt/axon/libaxon_pjrt.so")
    lib.axon_start_nrt_profile.argtypes = [ctypes.POINTER(ctypes.c_int64), ctypes.c_size_t]
    lib.axon_start_nrt_profile.restype = ctypes.c_int64
    lib.axon_stop_nrt_profile.argtypes = [ctypes.c_char_p]
    lib.axon_stop_nrt_profile.restype = ctypes.c_int64

    @contextlib.contextmanager
    def _hook(output_dir, device_ids):
        import jax
        jax.devices()
        if device_ids:
            ids = (ctypes.c_int64 * len(device_ids))(*device_ids)
            rc = lib.axon_start_nrt_profile(ids, len(device_ids))
        else:
            rc = lib.axon_start_nrt_profile(None, 0)
        if rc != 0:
            raise RuntimeError(f"axon_start_nrt_profile rc={rc}")
        try:
            yield
        finally:
            lib.axon_stop_nrt_profile(str(output_dir).encode())

    mod.set_axon_ntff_profile_hook(_hook)
    from concourse import bass_utils
    bass_utils.upload_artifacts = lambda tmpdir: str(tmpdir)


def _cdiv(a, b):
    return -(-a // b)


def _balance_pos(w):
    """Assign HALF dst nodes to NG groups of <=P lanes each, balancing total
    in-edge weight per group (greedy LPT).  Near-equal group loads on every
    core shrink the SPMD padding (slot counts are max-over-cores)."""
    import heapq
    order = np.argsort(-w, kind="stable")
    fill = np.zeros(NG, dtype=np.int64)
    pos = np.empty(w.shape[0], dtype=np.int64)
    hp = [(0.0, g) for g in range(NG)]
    heapq.heapify(hp)
    for d in order:
        while True:
            l, g = heapq.heappop(hp)
            if fill[g] < P:
                break
        pos[d] = g * P + fill[g]
        fill[g] += 1
        if fill[g] < P:
            heapq.heappush(hp, (l + float(w[d]), g))
    return pos


QCHOP = 768  # idxs per dma_gather call; 1008 (=63+1 descs/engine) corrupts data


def _big_gather(nc, data3, tbl, idx_t, col0, total, elem, blk0, qctr):
    """Single-packet dma_gather calls of <=QCHOP idxs covering `total` slots.
    Multi-packet big calls measured slower and NaN-flaky under profiling;
    chopped single-packet on rotated SWDGE queues is the best known config,
    with 768 (48+1 descs/engine) the measured sweet spot."""
    off = 0
    while off < total:
        n = min(QCHOP, total - off)
        b0 = blk0 + off // P
        nc.gpsimd.dma_gather(
            data3[:, b0:b0 + _cdiv(n, P), :], tbl[:],
            idx_t[:, col0 + off // 16:col0 + off // 16 + _cdiv(n, 16)],
            n, n, elem, single_packet=True,
            queue_num=qctr[0] % 4)
        qctr[0] += 1
        off += n


def _build_layout(cnt_lo, cnt_hi):
    """Static supertile layout shared by all 8 cores.

    cnt_lo/cnt_hi: [8, NG] per-core edge counts per (dst-group, table-kind).
    Returns per-supertile dicts with slot offsets, matmul lists, dmod/idx
    column bases.
    """
    SL = cnt_lo.max(axis=0).astype(np.int64)
    SH = cnt_hi.max(axis=0).astype(np.int64)
    sts = []
    m_base = 0
    ci_base = 0
    for t in range(NST):
        gs = list(range(G_PER * t, G_PER * (t + 1)))
        sL = [int(SL[g]) for g in gs]
        sH = [int(SH[g]) for g in gs]
        sumSL, sumSH = sum(sL), sum(sH)
        nbL, nbH = _cdiv(sumSL, P), _cdiv(sumSH, P)
        offL, offH = {}, {}
        o = 0
        for g, s in zip(gs, sL):
            offL[g] = o
            o += s
        o = 0
        for g, s in zip(gs, sH):
            offH[g] = o
            o += s
        mms = []  # (j_tile, g, kind, j_call)
        for kind in ("lo", "hi"):
            sumS = sumSL if kind == "lo" else sumSH
            nb = nbL if kind == "lo" else nbH
            offs = offL if kind == "lo" else offH
            S = sL if kind == "lo" else sH
            for j in range(nb):
                blk_a, blk_b = j * P, min((j + 1) * P, sumS)
                for gi, g in enumerate(gs):
                    a = offs[g]
                    b = a + S[gi]
                    if a < blk_b and b > blk_a and S[gi] > 0:
                        jt = j if kind == "lo" else nbL + j
                        mms.append((jt, g, kind, j))
        first, last = {}, {}
        for mi, (jt, g, kind, j) in enumerate(mms):
            first.setdefault(g, mi)
            last[g] = mi
        CL, CH = _cdiv(sumSL, 16), _cdiv(sumSH, 16)
        sts.append(dict(gs=gs, sL=sL, sH=sH, sumSL=sumSL, sumSH=sumSH,
                        nbL=nbL, nbH=nbH, offL=offL, offH=offH,
                        mms=mms, first=first, last=last,
                        m0=m_base, ci0=ci_base, CL=CL, CH=CH))
        m_base += len(mms)
        ci_base += CL + CH
    return SL, SH, sts, m_base, ci_base


def _core_tables(src_r, dst_l, sts, nm_tot, ic_tot):
    """Per-core idx (int16, [128, ic_tot]) and dmod (f32 -> bf16, [128, nm_tot])."""
    g = dst_l // P
    d = dst_l % P
    kindi = (src_r >= LO).astype(np.int64)
    order = np.lexsort((src_r, kindi, g))
    sg, sk, ss, sd = g[order], kindi[order], src_r[order], d[order]
    cnt = np.bincount(sg * 2 + sk, minlength=NG * 2)
    starts = np.concatenate([[0], np.cumsum(cnt)[:-1]]).reshape(NG, 2)
    cnt = cnt.reshape(NG, 2)

    idx16 = np.zeros((16, ic_tot), dtype=np.int16)
    dmod = np.full((P, nm_tot), 255.0, dtype=np.float32)
    for st in sts:
        dva_k = {}
        for kind in ("lo", "hi"):
            k = 0 if kind == "lo" else 1
            sumS = st["sumSL"] if kind == "lo" else st["sumSH"]
            nb = st["nbL"] if kind == "lo" else st["nbH"]
            offs = st["offL"] if kind == "lo" else st["offH"]
            ci = st["ci0"] if kind == "lo" else st["ci0"] + st["CL"]
            val = np.zeros(nb * P, dtype=np.int64)
            dva = np.full(nb * P, 255, dtype=np.int64)
            for g_ in st["gs"]:
                c = int(cnt[g_, k])
                s0 = int(starts[g_, k])
                a = offs[g_]
                if c:
                    val[a:a + c] = ss[s0:s0 + c] - (0 if kind == "lo" else LO)
                    dva[a:a + c] = sd[s0:s0 + c]
            if sumS:
                s_arr = np.arange(sumS)
                idx16[s_arr % 16, ci + s_arr // 16] = val[:sumS].astype(np.int16)
            dva_k[kind] = dva
        for mi, (jt, g_, kind, j) in enumerate(st["mms"]):
            dva = dva_k[kind]
            offs = st["offL"] if kind == "lo" else st["offH"]
            S = st["sL"] if kind == "lo" else st["sH"]
            gi = st["gs"].index(g_)
            a = offs[g_]
            b = a + S[gi]
            sl = j * P + np.arange(P)
            dmod[:, st["m0"] + mi] = np.where((sl >= a) & (sl < b), dva[sl], 255)
    return np.tile(idx16, (8, 1)), dmod


def _build_a(sts, nm_tot, ic_tot, nmm, nbmax, icmax, zb1=False):
    nc = bacc.Bacc("TRN2", target_bir_lowering=False, debug=False, num_devices=8,
                   num_swdge_queues=4)
    xT = nc.dram_tensor("xT", [F_IN, NP], BF16, kind="ExternalInput")
    W1 = nc.dram_tensor("W1", [F_IN, HID + OUT], BF16, kind="ExternalInput")
    W2 = nc.dram_tensor("W2", [HID, OUT], BF16, kind="ExternalInput")
    b1t_d = nc.dram_tensor("b1t", [P, HID], F32, kind="ExternalInput")
    dnod_d = nc.dram_tensor("dnod", [P, NCH], F32, kind="ExternalInput")
    ddst_d = nc.dram_tensor("ddst", [P, NG], F32, kind="ExternalInput")
    iota_d = nc.dram_tensor("iota3", [P, nmm, P], BF16, kind="ExternalInput")
    ident_d = nc.dram_tensor("ident", [P, P], BF16, kind="ExternalInput")
    dmod_d = nc.dram_tensor("dmod", [P, nm_tot], BF16, kind="ExternalInput")
    idx_d = nc.dram_tensor("idx", [P, ic_tot], I16, kind="ExternalInput")
    xw1_lo = nc.dram_tensor("xw1_lo", [LO, HID], BF16)
    xw1_hi = nc.dram_tensor("xw1_hi", [NHI, HID], BF16)
    xw2_o = nc.dram_tensor("xw2_own", [HP, OUT], BF16, kind="ExternalOutput")
    res_o = nc.dram_tensor("res_own", [HP, OUT], BF16, kind="ExternalOutput")

    ACT = mybir.ActivationFunctionType

    with tile.TileContext(nc) as tc:
        nc.gpsimd.load_library(library_config.mlp)
        with tc.tile_pool(name="const", bufs=1) as cp:
            w1a = cp.tile([P, HID + OUT], BF16); nc.sync.dma_start(out=w1a[:], in_=W1[0:P, :])
            w1b = cp.tile([P, HID + OUT], BF16); nc.sync.dma_start(out=w1b[:], in_=W1[P:2 * P, :])
            w2a = cp.tile([P, OUT], BF16); nc.sync.dma_start(out=w2a[:], in_=W2[0:P, :])
            w2b = cp.tile([P, OUT], BF16); nc.sync.dma_start(out=w2b[:], in_=W2[P:2 * P, :])
            b1t = cp.tile([P, HID], F32); nc.sync.dma_start(out=b1t[:], in_=b1t_d[:])
            dn_t = cp.tile([P, NCH], F32); nc.sync.dma_start(out=dn_t[:], in_=dnod_d[:])
            dd_t = cp.tile([P, NG], F32); nc.sync.dma_start(out=dd_t[:], in_=ddst_d[:])
            iota = cp.tile([P, nmm, P], BF16); nc.sync.dma_start(out=iota[:], in_=iota_d[:])
            ident = cp.tile([P, P], BF16); nc.sync.dma_start(out=ident[:], in_=ident_d[:])
            dmod_t = cp.tile([P, nm_tot], BF16); nc.sync.dma_start(out=dmod_t[:], in_=dmod_d[:])

            # step 1: xw1[n] = dinv[n] * (x[n] @ W1), bf16 tables; residual for own half
            # 8 chunks per iteration; one wide store per table / per res batch
            CB = 8
            with (
                tc.tile_pool(name="xt", bufs=4) as xtp,
                tc.tile_pool(name="mm1", bufs=4, space="PSUM") as mm1p,
                tc.tile_pool(name="sb1", bufs=3) as sb1p,
            ):
              for cc in range(NCH // CB):
                  xa = xtp.tile([P, CB * P], BF16, tag="xt")
                  nc.sync.dma_start(out=xa[:], in_=xT[0:P, cc * CB * P:(cc + 1) * CB * P])
                  xb = xtp.tile([P, CB * P], BF16, tag="xt")
                  nc.sync.dma_start(out=xb[:], in_=xT[P:2 * P, cc * CB * P:(cc + 1) * CB * P])
                  tw = sb1p.tile([P, CB, HID], BF16, tag="tw")
                  nres = min(max(NG - cc * CB, 0), CB)
                  rw = None
                  if nres:
                      rw = sb1p.tile([P, CB, OUT], BF16, tag="rw", name="rw")
                  for j in range(CB):
                      c = cc * CB + j
                      wid = HID + OUT if c < NG else HID
                      ps = mm1p.tile([P, HID + OUT], F32, tag="mm1")
                      nc.tensor.matmul(out=ps[:, 0:wid], lhsT=xa[:, j * P:(j + 1) * P],
                                       rhs=w1a[:, 0:wid], start=True, stop=False)
                      nc.tensor.matmul(out=ps[:, 0:wid], lhsT=xb[:, j * P:(j + 1) * P],
                                       rhs=w1b[:, 0:wid], start=False, stop=True)
                      if c % 2 == 0:
                          nc.scalar.activation(out=tw[:, j, :], in_=ps[:, 0:HID],
                                               func=ACT.Copy, scale=dn_t[:, c:c + 1])
                      else:
                          nc.vector.tensor_tensor(
                              out=tw[:, j, :], in0=dn_t[:, c:c + 1].to_broadcast([P, HID]),
                              in1=ps[:, 0:HID], op=mybir.AluOpType.mult)
                      if c < NG:
                          nc.vector.tensor_copy(out=rw[:, j, :], in_=ps[:, HID:HID + OUT])
                  if nres:
                      nc.scalar.dma_start(
                          out=res_o[cc * CB * P:(cc * CB + nres) * P, :]
                              .rearrange("(j p) f -> p j f", p=P),
                          in_=rw[:, 0:nres, :])
                  if cc < LO_CH // CB:
                      nc.sync.dma_start(
                          out=xw1_lo[cc * CB * P:(cc + 1) * CB * P, :]
                              .rearrange("(j p) f -> p j f", p=P),
                          in_=tw[:])
                  else:
                      cq = cc - LO_CH // CB
                      nc.sync.dma_start(
                          out=xw1_hi[cq * CB * P:(cq + 1) * CB * P, :]
                              .rearrange("(j p) f -> p j f", p=P),
                          in_=tw[:])

            tc.strict_bb_all_engine_barrier()

            # step 2: per-supertile gather + scatter-matmul + epilogue
            qctr = [0]
            with (
                tc.tile_pool(name="idx", bufs=3) as idxp,
                tc.tile_pool(name="data", bufs=4) as datap,
                tc.tile_pool(name="oh", bufs=4) as ohp,
                tc.tile_pool(name="agg", bufs=2 * G_PER, space="PSUM") as aggp,
                tc.tile_pool(name="tp", bufs=2, space="PSUM") as tpp,
                tc.tile_pool(name="mm2", bufs=2, space="PSUM") as mm2p,
                tc.tile_pool(name="ep", bufs=3) as epp,
            ):
              # clear the gather buffers: tail slots of partial last blocks are
              # never written by dma_gather; residual NaN there would poison the
              # one-hot matmuls (0*NaN=NaN) even under a zero one-hot column.
              for _ in range(4):
                  z = datap.tile([P, nbmax, HID], BF16, tag="data", name="zi")
                  nc.vector.memset(z[:], 0.0)
              for st in sts:
                  nbL, nbH = st["nbL"], st["nbH"]
                  nb = nbL + nbH
                  ict = st["CL"] + st["CH"]
                  nmt = len(st["mms"])
                  g0 = st["gs"][0]
                  idx_t = idxp.tile([P, icmax], I16, tag="idx")
                  nc.sync.dma_start(out=idx_t[:, 0:ict],
                                    in_=idx_d[:, st["ci0"]:st["ci0"] + ict])
                  data = datap.tile([P, nbmax, HID], BF16, tag="data")
                  if st["sumSL"]:
                      _big_gather(nc, data, xw1_lo, idx_t, 0,
                                  st["sumSL"], HID, 0, qctr)
                  if st["sumSH"]:
                      _big_gather(nc, data, xw1_hi, idx_t, st["CL"],
                                  st["sumSH"], HID, nbL, qctr)
                  slf2 = epp.tile([P, G_PER, HID], BF16, tag="slf")
                  nc.scalar.dma_start(
                      out=slf2[:],
                      in_=xw1_lo[g0 * P:(g0 + G_PER) * P, :]
                          .rearrange("(j p) f -> p j f", p=P))
                  xw2w = epp.tile([P, G_PER, OUT], BF16, tag="xw2w")
                  oh = ohp.tile([P, nmm, P], FP8, tag="oh")
                  nc.vector.tensor_tensor(
                      out=oh[:, 0:nmt, :],
                      in0=dmod_t[:, st["m0"]:st["m0"] + nmt].to_broadcast([P, nmt, P]),
                      in1=iota[:, 0:nmt, :], op=mybir.AluOpType.is_equal)
                  aggs = {}
                  for g in st["gs"]:
                      aggs[g] = aggp.tile([P, HID], F32, tag="agg", name=f"agg{g}")
                  for mi, (jt, g, kind, j) in enumerate(st["mms"]):
                      nc.tensor.matmul(out=aggs[g][:], lhsT=oh[:, mi, :],
                                       rhs=data[:, jt, :],
                                       start=(st["first"][g] == mi),
                                       stop=(st["last"][g] == mi))
                      if st["last"][g] != mi:
                          continue
                      # epilogue for group g
                      gi = st["gs"].index(g)
                      s = epp.tile([P, HID], F32, tag="s")
                      nc.vector.tensor_add(out=s[:], in0=aggs[g][:], in1=slf2[:, gi, :])
                      h = epp.tile([P, HID], BF16, tag="h")
                      if zb1:
                          # bias is all-zero: h = prelu(s * dinv_d) in one op
                          nc.scalar.activation(out=h[:], in_=s[:], func=ACT.Prelu,
                                               scale=dd_t[:, g:g + 1], alpha=0.2)
                      else:
                          s2 = epp.tile([P, HID], F32, tag="s2")
                          nc.scalar.activation(out=s2[:], in_=s[:], func=ACT.Copy,
                                               scale=dd_t[:, g:g + 1])
                          s3 = epp.tile([P, HID], F32, tag="s3")
                          nc.vector.tensor_add(out=s3[:], in0=s2[:], in1=b1t[:])
                          nc.scalar.activation(out=h[:], in_=s3[:], func=ACT.Prelu,
                                               alpha=0.2)
                      pt = tpp.tile([P, P], BF16, tag="pt")
                      nc.tensor.transpose(out=pt[:], in_=h[:, 0:P], identity=ident[:])
                      hta = epp.tile([P, P], BF16, tag="hta")
                      nc.vector.tensor_copy(out=hta[:], in_=pt[:])
                      pt2 = tpp.tile([P, P], BF16, tag="pt")
                      nc.tensor.transpose(out=pt2[:], in_=h[:, P:2 * P], identity=ident[:])
                      htb = epp.tile([P, P], BF16, tag="htb")
                      nc.vector.tensor_copy(out=htb[:], in_=pt2[:])
                      ps2 = mm2p.tile([P, OUT], F32, tag="mm2")
                      nc.tensor.matmul(out=ps2[:], lhsT=hta[:], rhs=w2a[:],
                                       start=True, stop=False)
                      nc.tensor.matmul(out=ps2[:], lhsT=htb[:], rhs=w2b[:],
                                       start=False, stop=True)
                      nc.scalar.activation(out=xw2w[:, gi, :], in_=ps2[:], func=ACT.Copy,
                                           scale=dd_t[:, g:g + 1])
                  nc.scalar.dma_start(
                      out=xw2_o[g0 * P:(g0 + G_PER) * P, :]
                          .rearrange("(j p) f -> p j f", p=P),
                      in_=xw2w[:])
    nc.compile()
    return nc


def _build_b(sts, nm_tot, ic_tot, nmm, nbmax, icmax, zb2=False):
    nc = bacc.Bacc("TRN2", target_bir_lowering=False, debug=False, num_devices=8,
                   num_swdge_queues=4)
    xw2_lo = nc.dram_tensor("xw2_lo", [LO, OUT], BF16, kind="ExternalInput")
    xw2_hi = nc.dram_tensor("xw2_hi", [NHI, OUT], BF16, kind="ExternalInput")
    b2t_d = nc.dram_tensor("b2t", [P, OUT], F32, kind="ExternalInput")
    ddst_d = nc.dram_tensor("ddst", [P, NG], F32, kind="ExternalInput")
    iota_d = nc.dram_tensor("iota3", [P, nmm, P], BF16, kind="ExternalInput")
    dmod_d = nc.dram_tensor("dmod", [P, nm_tot], BF16, kind="ExternalInput")
    idx_d = nc.dram_tensor("idx", [P, ic_tot], I16, kind="ExternalInput")
    out_o = nc.dram_tensor("out_own", [HP, OUT], BF16, kind="ExternalOutput")

    ACT = mybir.ActivationFunctionType

    with tile.TileContext(nc) as tc:
        nc.gpsimd.load_library(library_config.mlp)
        qctr = [0]
        with (
            tc.tile_pool(name="const", bufs=1) as cp,
            tc.tile_pool(name="idx", bufs=3) as idxp,
            tc.tile_pool(name="data", bufs=4) as datap,
            tc.tile_pool(name="oh", bufs=4) as ohp,
            tc.tile_pool(name="agg", bufs=6, space="PSUM") as aggp,
            tc.tile_pool(name="ep", bufs=3) as epp,
        ):
            b2t = cp.tile([P, OUT], F32); nc.sync.dma_start(out=b2t[:], in_=b2t_d[:])
            dd_t = cp.tile([P, NG], F32); nc.sync.dma_start(out=dd_t[:], in_=ddst_d[:])
            iota = cp.tile([P, nmm, P], BF16); nc.sync.dma_start(out=iota[:], in_=iota_d[:])
            dmod_t = cp.tile([P, nm_tot], BF16); nc.sync.dma_start(out=dmod_t[:], in_=dmod_d[:])

            # see _build_a: clear gather buffers against 0*NaN poisoning
            for _ in range(4):
                z = datap.tile([P, nbmax, OUT], BF16, tag="data", name="zi")
                nc.vector.memset(z[:], 0.0)
            for st in sts:
                nbL, nbH = st["nbL"], st["nbH"]
                nb = nbL + nbH
                ict = st["CL"] + st["CH"]
                nmt = len(st["mms"])
                g0 = st["gs"][0]
                idx_t = idxp.tile([P, icmax], I16, tag="idx")
                nc.sync.dma_start(out=idx_t[:, 0:ict],
                                  in_=idx_d[:, st["ci0"]:st["ci0"] + ict])
                data = datap.tile([P, nbmax, OUT], BF16, tag="data")
                if st["sumSL"]:
                    _big_gather(nc, data, xw2_lo, idx_t, 0,
                                st["sumSL"], OUT, 0, qctr)
                if st["sumSH"]:
                    _big_gather(nc, data, xw2_hi, idx_t, st["CL"],
                                st["sumSH"], OUT, nbL, qctr)
                slf2 = epp.tile([P, G_PER, OUT], BF16, tag="slf")
                nc.scalar.dma_start(
                    out=slf2[:],
                    in_=xw2_lo[g0 * P:(g0 + G_PER) * P, :]
                        .rearrange("(j p) f -> p j f", p=P))
                ow = epp.tile([P, G_PER, OUT], BF16, tag="ow")
                oh = ohp.tile([P, nmm, P], FP8, tag="oh")
                nc.vector.tensor_tensor(
                    out=oh[:, 0:nmt, :],
                    in0=dmod_t[:, st["m0"]:st["m0"] + nmt].to_broadcast([P, nmt, P]),
                    in1=iota[:, 0:nmt, :], op=mybir.AluOpType.is_equal)
                aggs = {}
                for g in st["gs"]:
                    aggs[g] = aggp.tile([P, OUT], F32, tag="agg", name=f"agg{g}")
                for mi, (jt, g, kind, j) in enumerate(st["mms"]):
                    nc.tensor.matmul(out=aggs[g][:], lhsT=oh[:, mi, :],
                                     rhs=data[:, jt, :],
                                     start=(st["first"][g] == mi),
                                     stop=(st["last"][g] == mi))
                    if st["last"][g] != mi:
                        continue
                    gi = st["gs"].index(g)
                    s = epp.tile([P, OUT], F32, tag="s")
                    nc.vector.tensor_add(out=s[:], in0=aggs[g][:], in1=slf2[:, gi, :])
                    if zb2:
                        nc.scalar.activation(out=ow[:, gi, :], in_=s[:], func=ACT.Prelu,
                                             scale=dd_t[:, g:g + 1], alpha=0.2)
                    else:
                        s2 = epp.tile([P, OUT], F32, tag="s2")
                        nc.scalar.activation(out=s2[:], in_=s[:], func=ACT.Copy,
                                             scale=dd_t[:, g:g + 1])
                        s3 = epp.tile([P, OUT], F32, tag="s3")
                        nc.vector.tensor_add(out=s3[:], in0=s2[:], in1=b2t[:])
                        nc.scalar.activation(out=ow[:, gi, :], in_=s3[:], func=ACT.Prelu,
                                             alpha=0.2)
                nc.scalar.dma_start(
                    out=out_o[g0 * P:(g0 + G_PER) * P, :]
                        .rearrange("(j p) f -> p j f", p=P),
                    in_=ow[:])
    nc.compile()
    return nc


def kernel(**inputs):
    trace = os.environ.get("KERNEL_TRACE", "0") == "1"
    if trace:
        try:
            _install_ntff_hook()
        except Exception:
            trace = False

    f32 = np.float32
    stacks = [
        ("x_lnc_jac", "edge_jac_lnc", "W_j1_lnc", "b_j1_lnc", "W_j2_lnc", "b_j2_lnc", "W_res_lnc"),
        ("x_prot_jac", "edge_jac_prot", "W_j1_prot", "b_j1_prot", "W_j2_prot", "b_j2_prot", "W_res_prot"),
        ("x_lnc_blast", "edge_blast_lnc", "W_b1_lnc", "b_b1_lnc", "W_b2_lnc", "b_b2_lnc", None),
        ("x_prot_blast", "edge_blast_prot", "W_b1_prot", "b_b1_prot", "W_b2_prot", "b_b2_prot", None),
    ]

    # per-core edge preprocessing.  Pass 1: per-core balanced dst->row
    # permutation (equalizes per-group edge counts so the SPMD max-over-cores
    # slot padding nearly vanishes).  Pass 2: relabel edges through the
    # row maps (partner srcs use the partner core's permutation).
    raw = []
    pos_owns = []
    for c in range(8):
        sname = stacks[c // 2]
        h = c % 2
        e = np.asarray(inputs[sname[1]])
        src, dst = e[0].astype(np.int64), e[1].astype(np.int64)
        deg = (np.bincount(dst, minlength=N) + 1.0).astype(f32)
        dinv = (1.0 / np.sqrt(deg)).astype(f32)
        sel = (dst >= h * HALF) & (dst < (h + 1) * HALF)
        srcs, dsts = src[sel], dst[sel] - h * HALF
        w = np.bincount(dsts, minlength=HALF)
        pos_owns.append(_balance_pos(w))
        raw.append((srcs, dsts, dinv))

    pre = []
    cnt_lo = np.zeros((8, NG), dtype=np.int64)
    cnt_hi = np.zeros((8, NG), dtype=np.int64)
    for c in range(8):
        h = c % 2
        srcs, dsts, dinv = raw[c]
        pos_own, pos_par = pos_owns[c], pos_owns[c ^ 1]
        own = (srcs >= h * HALF) & (srcs < (h + 1) * HALF)
        src_r = np.where(own, pos_own[np.clip(srcs - h * HALF, 0, HALF - 1)],
                         HP + pos_par[np.clip(srcs - (1 - h) * HALF, 0, HALF - 1)])
        dst_l = pos_own[dsts]
        gi = dst_l // P
        ki = (src_r >= LO).astype(np.int64)
        cnt_lo[c] = np.bincount(gi[ki == 0], minlength=NG)
        cnt_hi[c] = np.bincount(gi[ki == 1], minlength=NG)
        pre.append((src_r, dst_l, dinv))

    SL, SH, sts, nm_tot, ic_tot = _build_layout(cnt_lo, cnt_hi)
    if os.environ.get("KERNEL_DEBUG", "0") == "1":
        slots = int(SL.sum() + SH.sum())
        print(f"dbg slots={slots} edges/core~{cnt_lo.sum(1).mean()+cnt_hi.sum(1).mean():.0f} "
              f"pad={slots / (cnt_lo.sum(1).mean()+cnt_hi.sum(1).mean()) - 1:.4f}",
              flush=True)
    nmm = max(len(st["mms"]) for st in sts)
    nbmax = max(st["nbL"] + st["nbH"] for st in sts)
    icmax = max(st["CL"] + st["CH"] for st in sts)

    iota3 = np.broadcast_to(np.arange(P, dtype=f32), (P, nmm, P)).astype(BF).copy()
    ident = np.eye(P, dtype=f32).astype(BF)

    in_a, in_b_partial = [], []
    for c in range(8):
        sname = stacks[c // 2]
        h = c % 2
        src_r, dst_l, dinv = pre[c]
        idx_full, dmod = _core_tables(src_r, dst_l, sts, nm_tot, ic_tot)
        x = np.asarray(inputs[sname[0]], dtype=f32)
        nodeatrow = np.full(NP, -1, dtype=np.int64)
        nodeatrow[pos_owns[c]] = h * HALF + np.arange(HALF)
        nodeatrow[HP + pos_owns[c ^ 1]] = (1 - h) * HALF + np.arange(HALF)
        valid = nodeatrow >= 0
        xT = np.zeros((F_IN, NP), dtype=f32)
        xT[:, valid] = x[nodeatrow[valid]].T
        dpad = np.ones(NP, dtype=f32)
        dpad[valid] = dinv[nodeatrow[valid]]
        dnod = dpad[np.arange(NP, dtype=np.int64).reshape(NCH, P).T].astype(f32)
        ddst = dpad[np.arange(HP, dtype=np.int64).reshape(NG, P).T].astype(f32)
        W1 = np.asarray(inputs[sname[2]], dtype=f32)
        b1 = np.asarray(inputs[sname[3]], dtype=f32)
        W2 = np.asarray(inputs[sname[4]], dtype=f32)
        b2 = np.asarray(inputs[sname[5]], dtype=f32)
        Wr = np.asarray(inputs[sname[6]], dtype=f32) if sname[6] else np.zeros((F_IN, OUT), dtype=f32)
        dmod_bf = dmod.astype(BF)
        xT_bf = xT.astype(BF)
        in_a.append({
            "xT": xT_bf, "W1": np.concatenate([W1, Wr], axis=1).astype(BF),
            "W2": W2.astype(BF),
            "b1t": np.broadcast_to(b1, (P, HID)).copy(),
            "dnod": np.ascontiguousarray(dnod), "ddst": np.ascontiguousarray(ddst),
            "iota3": iota3, "ident": ident,
            "dmod": dmod_bf, "idx": idx_full,
        })
        in_b_partial.append({
            "b2t": np.broadcast_to(b2, (P, OUT)).copy(),
            "ddst": np.ascontiguousarray(ddst),
            "iota3": iota3, "dmod": dmod_bf, "idx": idx_full,
        })

    import tempfile
    zb1 = all(not np.any(np.asarray(inputs[stacks[s][3]])) for s in range(4))
    zb2 = all(not np.any(np.asarray(inputs[stacks[s][5]])) for s in range(4))
    nc_a = _build_a(sts, nm_tot, ic_tot, nmm, nbmax, icmax, zb1=zb1)
    res_a = run_bass_kernel_spmd(nc_a, in_a, list(range(8)), trace=trace,
                                 tmpdir=tempfile.mkdtemp(prefix="gnn_a_") if trace else None)
    LAST_EXEC_NS.clear()
    if trace and res_a.exec_time_ns:
        LAST_EXEC_NS.append(res_a.exec_time_ns)

    if os.environ.get("KERNEL_DEBUG", "0") == "1":
        for c in range(8):
            xo = np.asarray(res_a.results[c]["xw2_own"]).astype(np.float32)
            ro = np.asarray(res_a.results[c]["res_own"]).astype(np.float32)
            print(f"dbg core {c}: xw2 nan%={np.isnan(xo).mean():.4f} "
                  f"max={np.nanmax(np.abs(xo)):.3f}  res nan%={np.isnan(ro).mean():.4f} "
                  f"max={np.nanmax(np.abs(ro)):.3f}", flush=True)
        xo = np.asarray(res_a.results[0]["xw2_own"]).astype(np.float32)
        nanrow = np.isnan(xo).any(axis=1)
        print("dbg nan rows core0:", np.where(nanrow)[0][:20], nanrow.sum(), flush=True)

    # host halo exchange: assemble full xw2 per pair (bf16 concat only;
    # both halves are already in their core's permuted row order)
    in_b = []
    for c in range(8):
        partner = c ^ 1
        xw2f = np.zeros((NP, OUT), dtype=BF)
        xw2f[:HP] = np.asarray(res_a.results[c]["xw2_own"])
        xw2f[HP:] = np.asarray(res_a.results[partner]["xw2_own"])
        in_b.append({"xw2_lo": xw2f[:LO].copy(), "xw2_hi": xw2f[LO:].copy(),
                     **in_b_partial[c]})

    nc_b = _build_b(sts, nm_tot, ic_tot, nmm, nbmax, icmax, zb2=zb2)
    res_b = run_bass_kernel_spmd(nc_b, in_b, list(range(8)), trace=trace,
                                 tmpdir=tempfile.mkdtemp(prefix="gnn_b_") if trace else None)
    if trace and res_b.exec_time_ns:
        LAST_EXEC_NS.append(res_b.exec_time_ns)

    def full_out(pair):
        return np.concatenate([
            np.asarray(res_b.results[2 * pair]["out_own"])[pos_owns[2 * pair]],
            np.asarray(res_b.results[2 * pair + 1]["out_own"])[pos_owns[2 * pair + 1]],
        ]).astype(f32)

    jl, jp, bl, bp = full_out(0), full_out(1), full_out(2), full_out(3)
    res_l = np.concatenate(
        [np.asarray(res_a.results[0]["res_own"])[pos_owns[0]],
         np.asarray(res_a.results[1]["res_own"])[pos_owns[1]]]).astype(f32) \
        + np.asarray(inputs["b_res_lnc"], dtype=f32)
    res_p = np.concatenate(
        [np.asarray(res_a.results[2]["res_own"])[pos_owns[2]],
         np.asarray(res_a.results[3]["res_own"])[pos_owns[3]]]).astype(f32) \
        + np.asarray(inputs["b_res_prot"], dtype=f32)
    comb_l = (jl + bl) * 0.5 + res_l
    comb_p = (jp + bp) * 0.5 + res_p
    return (comb_l, comb_p, jl, jp, bl, bp)



# revision 46
# speedup vs baseline: 1.3507x; 1.0289x over previous
"""HeteroGNN (2-layer GCN x 4 stacks) on 8 Trainium2 NeuronCores.

Sharding: cores {2s, 2s+1} handle stack s (jac-lnc, jac-prot, blast-lnc,
blast-prot); within a pair, destination nodes are split in halves of 25000.
Each core's 25000 dst nodes are assigned to its 196 dst-groups by a greedy
balanced (LPT) permutation of in-edge counts, so per-(group, table) slot
counts are near-identical across the 8 SPMD cores and the max-over-cores
padding shrinks from ~12.5% to ~3%.  Outputs are unpermuted on the host.

Per GCN layer, transform-first: the gather table holds dinv-prescaled
transformed rows in bf16, split into a "lo" table (rows < 32768) and a "hi"
table so row ids fit int16 for dma_gather.  Edges are bucketed by
(dst-group, lo/hi) on the host; per 2-group supertile the kernel issues
single-packet dma_gather calls of <=768 idxs on 4 rotated SWDGE queues,
builds all one-hot scatter matrices with a single wide is_equal, and
accumulates per-dst-group aggregates in PSUM via bf16 one-hot matmuls.
The dense phase-1 transform batches 8 node-chunks per wide DMA
(store via rearranged APs) and alternates HWDGE issue between the Sync and
Scalar sequencers; epilogue loads/stores are likewise batched per supertile.
Gather data buffers are memset once at startup: tail slots of partial last
blocks are never written by dma_gather, and residual NaN there would poison
the one-hot matmuls through 0*NaN even under a zero one-hot column.
The inter-layer halo exchange (pair halves of xw2) goes through the host
between two SPMD launches; the final view-combine is elementwise on host.

Measured on trn2 (8 cores): ~2.56-2.68 ms total HW exec (launch A ~1.47 ms
+ launch B ~1.11 ms; run-to-run spread +-60 us), rel-err ~5.4e-3.
Load-bearing tuning: QCHOP=768 (48+1 descs/engine) is a sharp optimum
(512: +0.12 ms, 640: +0.24 ms, 896: +0.37 ms, 1008: CORRUPT), per-call
SWDGE queue rotation qctr%4 (pairing calls per queue costs +0.8 ms),
datap bufs=4 (3 costs ~0.17 ms: the
gather instruction holds the GpSimd engine while waiting out the data-tile
WAR on matmuls three supertiles back), ohp bufs=4, idxp/epp bufs=3
(4 costs ~0.2 ms).  One-hot is fp8e4 (exact 0/1; mixed fp8xbf16 matmul is
bit-identical to bf16 here) and the epilogue fuses scale+prelu into one
scalar activation when the layer biases are all zero (checked at build
time).  Known-bad variants (do not retry blindly): multi-packet big
gathers (NaN-flaky + slower), whole-run idx prefetch (+0.5 ms), bigger
SWDGE ring, fp8 gather tables (rel-err 2e-2, at the gate), GpSimd
scatter_add/ap_gather data-plane aggregation (~37 ns/idx, duplicates
dropped).  Launch B is bound by the per-supertile chain of gather-drain
completion semaphores (8 DMAHW lanes) + IS_EQ (4.9 us, input-bound: fp8
output does not speed it up) + one-hot matmuls; Q7 descriptor emission
itself is ~0.2 ns/idx when not blocked.
"""
import os
import sys

sys.path.insert(0, "/opt/trn_rl_repo")

import numpy as np
import ml_dtypes

import concourse.bass as bass
import concourse.mybir as mybir
import concourse.tile as tile
from concourse import bacc
from concourse import library_config
from concourse.bass_utils import run_bass_kernel_spmd

N = 50000
NP = 50176          # padded (392 * 128)
HALF = 25000
HP = 25088          # padded half (196 * 128)
NG = 196            # dst groups per half
NCH = 392           # node chunks for the dense transform
F_IN = 256
HID = 256
OUT = 128
P = 128
LO = 32768          # rows in the "lo" gather table (int16 range)
NHI = NP - LO       # 17408
LO_CH = LO // P     # 256 node chunks go to the lo table
G_PER = 2           # dst groups per supertile
NST = NG // G_PER   # 98 supertiles

F32 = mybir.dt.float32
BF16 = mybir.dt.bfloat16
FP8 = mybir.dt.float8e4
I16 = mybir.dt.int16
BF = ml_dtypes.bfloat16

LAST_EXEC_NS = []   # filled when KERNEL_TRACE=1


def _install_ntff_hook():
    """Register the axon NTFF profile hook (the image's antenv lacks it) and
    neuter the S3 artifact upload so tracing works offline."""
    import types, contextlib, ctypes
    import antenv  # noqa: F401
    mod = types.ModuleType("antenv.axon_hooks")
    holder = {"hook": None}
    mod.set_axon_ntff_profile_hook = lambda h: holder.__setitem__("hook", h)
    mod.get_axon_ntff_profile_hook = lambda: holder["hook"]
    sys.modules["antenv.axon_hooks"] = mod
    lib = ctypes.CDLL("/opt/axon/libaxon_pjrt.so")
    lib.axon_start_nrt_profile.argtypes = [ctypes.POINTER(ctypes.c_int64), ctypes.c_size_t]
    lib.axon_start_nrt_profile.restype = ctypes.c_int64
    lib.axon_stop_nrt_profile.argtypes = [ctypes.c_char_p]
    lib.axon_stop_nrt_profile.restype = ctypes.c_int64

    @contextlib.contextmanager
    def _hook(output_dir, device_ids):
        import jax
        jax.devices()
        if device_ids:
            ids = (ctypes.c_int64 * len(device_ids))(*device_ids)
            rc = lib.axon_start_nrt_profile(ids, len(device_ids))
        else:
            rc = lib.axon_start_nrt_profile(None, 0)
        if rc != 0:
            raise RuntimeError(f"axon_start_nrt_profile rc={rc}")
        try:
            yield
        finally:
            lib.axon_stop_nrt_profile(str(output_dir).encode())

    mod.set_axon_ntff_profile_hook(_hook)
    from concourse import bass_utils
    bass_utils.upload_artifacts = lambda tmpdir: str(tmpdir)


def _cdiv(a, b):
    return -(-a // b)


def _balance_pos(w):
    """Assign HALF dst nodes to NG groups of <=P lanes each, balancing total
    in-edge weight per group (greedy LPT).  Near-equal group loads on every
    core shrink the SPMD padding (slot counts are max-over-cores)."""
    import heapq
    order = np.argsort(-w, kind="stable")
    fill = np.zeros(NG, dtype=np.int64)
    pos = np.empty(w.shape[0], dtype=np.int64)
    hp = [(0.0, g) for g in range(NG)]
    heapq.heapify(hp)
    for d in order:
        while True:
            l, g = heapq.heappop(hp)
            if fill[g] < P:
                break
        pos[d] = g * P + fill[g]
        fill[g] += 1
        if fill[g] < P:
            heapq.heappush(hp, (l + float(w[d]), g))
    return pos


QCHOP = 768  # idxs per dma_gather call; 1008 (=63+1 descs/engine) corrupts data


def _big_gather(nc, data3, tbl, idx_t, col0, total, elem, blk0, qctr):
    """Single-packet dma_gather calls of <=QCHOP idxs covering `total` slots.
    Multi-packet big calls measured slower and NaN-flaky under profiling;
    chopped single-packet on rotated SWDGE queues is the best known config,
    with 768 (48+1 descs/engine) the measured sweet spot."""
    off = 0
    while off < total:
        n = min(QCHOP, total - off)
        b0 = blk0 + off // P
        nc.gpsimd.dma_gather(
            data3[:, b0:b0 + _cdiv(n, P), :], tbl[:],
            idx_t[:, col0 + off // 16:col0 + off // 16 + _cdiv(n, 16)],
            n, n, elem, single_packet=True,
            queue_num=qctr[0] % 4)
        qctr[0] += 1
        off += n


def _build_layout(cnt_lo, cnt_hi):
    """Static supertile layout shared by all 8 cores.

    cnt_lo/cnt_hi: [8, NG] per-core edge counts per (dst-group, table-kind).
    Returns per-supertile dicts with slot offsets, matmul lists, dmod/idx
    column bases.
    """
    SL = cnt_lo.max(axis=0).astype(np.int64)
    SH = cnt_hi.max(axis=0).astype(np.int64)
    sts = []
    m_base = 0
    ci_base = 0
    for t in range(NST):
        gs = list(range(G_PER * t, G_PER * (t + 1)))
        sL = [int(SL[g]) for g in gs]
        sH = [int(SH[g]) for g in gs]
        sumSL, sumSH = sum(sL), sum(sH)
        nbL, nbH = _cdiv(sumSL, P), _cdiv(sumSH, P)
        offL, offH = {}, {}
        o = 0
        for g, s in zip(gs, sL):
            offL[g] = o
            o += s
        o = 0
        for g, s in zip(gs, sH):
            offH[g] = o
            o += s
        mms = []  # (j_tile, g, kind, j_call)
        for kind in ("lo", "hi"):
            sumS = sumSL if kind == "lo" else sumSH
            nb = nbL if kind == "lo" else nbH
            offs = offL if kind == "lo" else offH
            S = sL if kind == "lo" else sH
            for j in range(nb):
                blk_a, blk_b = j * P, min((j + 1) * P, sumS)
                for gi, g in enumerate(gs):
                    a = offs[g]
                    b = a + S[gi]
                    if a < blk_b and b > blk_a and S[gi] > 0:
                        jt = j if kind == "lo" else nbL + j
                        mms.append((jt, g, kind, j))
        first, last = {}, {}
        for mi, (jt, g, kind, j) in enumerate(mms):
            first.setdefault(g, mi)
            last[g] = mi
        CL, CH = _cdiv(sumSL, 16), _cdiv(sumSH, 16)
        sts.append(dict(gs=gs, sL=sL, sH=sH, sumSL=sumSL, sumSH=sumSH,
                        nbL=nbL, nbH=nbH, offL=offL, offH=offH,
                        mms=mms, first=first, last=last,
                        m0=m_base, ci0=ci_base, CL=CL, CH=CH))
        m_base += len(mms)
        ci_base += CL + CH
    return SL, SH, sts, m_base, ci_base


def _core_tables(src_r, dst_l, sts, nm_tot, ic_tot):
    """Per-core idx (int16, [128, ic_tot]) and dmod (f32 -> bf16, [128, nm_tot])."""
    g = dst_l // P
    d = dst_l % P
    kindi = (src_r >= LO).astype(np.int64)
    order = np.lexsort((src_r, kindi, g))
    sg, sk, ss, sd = g[order], kindi[order], src_r[order], d[order]
    cnt = np.bincount(sg * 2 + sk, minlength=NG * 2)
    starts = np.concatenate([[0], np.cumsum(cnt)[:-1]]).reshape(NG, 2)
    cnt = cnt.reshape(NG, 2)

    idx16 = np.zeros((16, ic_tot), dtype=np.int16)
    dmod = np.full((P, nm_tot), 255.0, dtype=np.float32)
    for st in sts:
        dva_k = {}
        for kind in ("lo", "hi"):
            k = 0 if kind == "lo" else 1
            sumS = st["sumSL"] if kind == "lo" else st["sumSH"]
            nb = st["nbL"] if kind == "lo" else st["nbH"]
            offs = st["offL"] if kind == "lo" else st["offH"]
            ci = st["ci0"] if kind == "lo" else st["ci0"] + st["CL"]
            val = np.zeros(nb * P, dtype=np.int64)
            dva = np.full(nb * P, 255, dtype=np.int64)
            for g_ in st["gs"]:
                c = int(cnt[g_, k])
                s0 = int(starts[g_, k])
                a = offs[g_]
                if c:
                    val[a:a + c] = ss[s0:s0 + c] - (0 if kind == "lo" else LO)
                    dva[a:a + c] = sd[s0:s0 + c]
            if sumS:
                s_arr = np.arange(sumS)
                idx16[s_arr % 16, ci + s_arr // 16] = val[:sumS].astype(np.int16)
            dva_k[kind] = dva
        for mi, (jt, g_, kind, j) in enumerate(st["mms"]):
            dva = dva_k[kind]
            offs = st["offL"] if kind == "lo" else st["offH"]
            S = st["sL"] if kind == "lo" else st["sH"]
            gi = st["gs"].index(g_)
            a = offs[g_]
            b = a + S[gi]
            sl = j * P + np.arange(P)
            dmod[:, st["m0"] + mi] = np.where((sl >= a) & (sl < b), dva[sl], 255)
    return np.tile(idx16, (8, 1)), dmod


def _build_a(sts, nm_tot, ic_tot, nmm, nbmax, icmax, zb1=False):
    nc = bacc.Bacc("TRN2", target_bir_lowering=False, debug=False, num_devices=8,
                   num_swdge_queues=4)
    xT = nc.dram_tensor("xT", [F_IN, NP], BF16, kind="ExternalInput")
    W1 = nc.dram_tensor("W1", [F_IN, HID + OUT], BF16, kind="ExternalInput")
    W2 = nc.dram_tensor("W2", [HID, OUT], BF16, kind="ExternalInput")
    b1t_d = nc.dram_tensor("b1t", [P, HID], F32, kind="ExternalInput")
    dnod_d = nc.dram_tensor("dnod", [P, NCH], F32, kind="ExternalInput")
    ddst_d = nc.dram_tensor("ddst", [P, NG], F32, kind="ExternalInput")
    iota_d = nc.dram_tensor("iota3", [P, nmm, P], BF16, kind="ExternalInput")
    ident_d = nc.dram_tensor("ident", [P, P], BF16, kind="ExternalInput")
    dmod_d = nc.dram_tensor("dmod", [P, nm_tot], BF16, kind="ExternalInput")
    idx_d = nc.dram_tensor("idx", [P, ic_tot], I16, kind="ExternalInput")
    xw1_lo = nc.dram_tensor("xw1_lo", [LO, HID], BF16)
    xw1_hi = nc.dram_tensor("xw1_hi", [NHI, HID], BF16)
    xw2_o = nc.dram_tensor("xw2_own", [HP, OUT], BF16, kind="ExternalOutput")
    res_o = nc.dram_tensor("res_own", [HP, OUT], BF16, kind="ExternalOutput")

    ACT = mybir.ActivationFunctionType

    with tile.TileContext(nc) as tc:
        nc.gpsimd.load_library(library_config.mlp)
        with tc.tile_pool(name="const", bufs=1) as cp:
            w1a = cp.tile([P, HID + OUT], BF16); nc.sync.dma_start(out=w1a[:], in_=W1[0:P, :])
            w1b = cp.tile([P, HID + OUT], BF16); nc.sync.dma_start(out=w1b[:], in_=W1[P:2 * P, :])
            w2a = cp.tile([P, OUT], BF16); nc.sync.dma_start(out=w2a[:], in_=W2[0:P, :])
            w2b = cp.tile([P, OUT], BF16); nc.sync.dma_start(out=w2b[:], in_=W2[P:2 * P, :])
            b1t = cp.tile([P, HID], F32); nc.sync.dma_start(out=b1t[:], in_=b1t_d[:])
            dn_t = cp.tile([P, NCH], F32); nc.sync.dma_start(out=dn_t[:], in_=dnod_d[:])
            dd_t = cp.tile([P, NG], F32); nc.sync.dma_start(out=dd_t[:], in_=ddst_d[:])
            iota = cp.tile([P, nmm, P], BF16); nc.sync.dma_start(out=iota[:], in_=iota_d[:])
            ident = cp.tile([P, P], BF16); nc.sync.dma_start(out=ident[:], in_=ident_d[:])
            dmod_t = cp.tile([P, nm_tot], BF16); nc.sync.dma_start(out=dmod_t[:], in_=dmod_d[:])

            # step 1: xw1[n] = dinv[n] * (x[n] @ W1), bf16 tables; residual for own half
            # 8 chunks per iteration; one wide store per table / per res batch
            CB = 8
            with (
                tc.tile_pool(name="xt", bufs=4) as xtp,
                tc.tile_pool(name="mm1", bufs=4, space="PSUM") as mm1p,
                tc.tile_pool(name="sb1", bufs=3) as sb1p,
            ):
              for cc in range(NCH // CB):
                  xa = xtp.tile([P, CB * P], BF16, tag="xt")
                  nc.sync.dma_start(out=xa[:], in_=xT[0:P, cc * CB * P:(cc + 1) * CB * P])
                  xb = xtp.tile([P, CB * P], BF16, tag="xt")
                  nc.sync.dma_start(out=xb[:], in_=xT[P:2 * P, cc * CB * P:(cc + 1) * CB * P])
                  tw = sb1p.tile([P, CB, HID], BF16, tag="tw")
                  nres = min(max(NG - cc * CB, 0), CB)
                  rw = None
                  if nres:
                      rw = sb1p.tile([P, CB, OUT], BF16, tag="rw", name="rw")
                  for j in range(CB):
                      c = cc * CB + j
                      wid = HID + OUT if c < NG else HID
                      ps = mm1p.tile([P, HID + OUT], F32, tag="mm1")
                      nc.tensor.matmul(out=ps[:, 0:wid], lhsT=xa[:, j * P:(j + 1) * P],
                                       rhs=w1a[:, 0:wid], start=True, stop=False)
                      nc.tensor.matmul(out=ps[:, 0:wid], lhsT=xb[:, j * P:(j + 1) * P],
                                       rhs=w1b[:, 0:wid], start=False, stop=True)
                      if c % 2 == 0:
                          nc.scalar.activation(out=tw[:, j, :], in_=ps[:, 0:HID],
                                               func=ACT.Copy, scale=dn_t[:, c:c + 1])
                      else:
                          nc.vector.tensor_tensor(
                              out=tw[:, j, :], in0=dn_t[:, c:c + 1].to_broadcast([P, HID]),
                              in1=ps[:, 0:HID], op=mybir.AluOpType.mult)
                      if c < NG:
                          nc.vector.tensor_copy(out=rw[:, j, :], in_=ps[:, HID:HID + OUT])
                  if nres:
                      nc.scalar.dma_start(
                          out=res_o[cc * CB * P:(cc * CB + nres) * P, :]
                              .rearrange("(j p) f -> p j f", p=P),
                          in_=rw[:, 0:nres, :])
                  if cc < LO_CH // CB:
                      nc.sync.dma_start(
                          out=xw1_lo[cc * CB * P:(cc + 1) * CB * P, :]
                              .rearrange("(j p) f -> p j f", p=P),
                          in_=tw[:])
                  else:
                      cq = cc - LO_CH // CB
                      nc.sync.dma_start(
                          out=xw1_hi[cq * CB * P:(cq + 1) * CB * P, :]
                              .rearrange("(j p) f -> p j f", p=P),
                          in_=tw[:])

            tc.strict_bb_all_engine_barrier()

            # step 2: per-supertile gather + scatter-matmul + epilogue
            qctr = [0]
            with (
                tc.tile_pool(name="idx", bufs=3) as idxp,
                tc.tile_pool(name="data", bufs=4) as datap,
                tc.tile_pool(name="oh", bufs=4) as ohp,
                tc.tile_pool(name="agg", bufs=2 * G_PER, space="PSUM") as aggp,
                tc.tile_pool(name="tp", bufs=2, space="PSUM") as tpp,
                tc.tile_pool(name="mm2", bufs=2, space="PSUM") as mm2p,
                tc.tile_pool(name="ep", bufs=3) as epp,
            ):
              # clear the gather buffers: tail slots of partial last blocks are
              # never written by dma_gather; residual NaN there would poison the
              # one-hot matmuls (0*NaN=NaN) even under a zero one-hot column.
              for _ in range(4):
                  z = datap.tile([P, nbmax, HID], BF16, tag="data", name="zi")
                  nc.vector.memset(z[:], 0.0)
              for st in sts:
                  nbL, nbH = st["nbL"], st["nbH"]
                  nb = nbL + nbH
                  ict = st["CL"] + st["CH"]
                  nmt = len(st["mms"])
                  g0 = st["gs"][0]
                  idx_t = idxp.tile([P, icmax], I16, tag="idx")
                  nc.sync.dma_start(out=idx_t[:, 0:ict],
                                    in_=idx_d[:, st["ci0"]:st["ci0"] + ict])
                  data = datap.tile([P, nbmax, HID], BF16, tag="data")
                  if st["sumSL"]:
                      _big_gather(nc, data, xw1_lo, idx_t, 0,
                                  st["sumSL"], HID, 0, qctr)
                  if st["sumSH"]:
                      _big_gather(nc, data, xw1_hi, idx_t, st["CL"],
                                  st["sumSH"], HID, nbL, qctr)
                  slf2 = epp.tile([P, G_PER, HID], BF16, tag="slf")
                  nc.scalar.dma_start(
                      out=slf2[:],
                      in_=xw1_lo[g0 * P:(g0 + G_PER) * P, :]
                          .rearrange("(j p) f -> p j f", p=P))
                  xw2w = epp.tile([P, G_PER, OUT], BF16, tag="xw2w")
                  oh = ohp.tile([P, nmm, P], FP8, tag="oh")
                  h1 = max(nmt // 2, 1)
                  nc.vector.tensor_tensor(
                      out=oh[:, 0:h1, :],
                      in0=dmod_t[:, st["m0"]:st["m0"] + h1].to_broadcast([P, h1, P]),
                      in1=iota[:, 0:h1, :], op=mybir.AluOpType.is_equal)
                  if nmt > h1:
                      nc.vector.tensor_tensor(
                          out=oh[:, h1:nmt, :],
                          in0=dmod_t[:, st["m0"] + h1:st["m0"] + nmt]
                              .to_broadcast([P, nmt - h1, P]),
                          in1=iota[:, 0:nmt - h1, :], op=mybir.AluOpType.is_equal)
                  aggs = {}
                  for g in st["gs"]:
                      aggs[g] = aggp.tile([P, HID], F32, tag="agg", name=f"agg{g}")
                  for mi, (jt, g, kind, j) in enumerate(st["mms"]):
                      nc.tensor.matmul(out=aggs[g][:], lhsT=oh[:, mi, :],
                                       rhs=data[:, jt, :],
                                       start=(st["first"][g] == mi),
                                       stop=(st["last"][g] == mi))
                      if st["last"][g] != mi:
                          continue
                      # epilogue for group g
                      gi = st["gs"].index(g)
                      s = epp.tile([P, HID], F32, tag="s")
                      nc.vector.tensor_add(out=s[:], in0=aggs[g][:], in1=slf2[:, gi, :])
                      h = epp.tile([P, HID], BF16, tag="h")
                      if zb1:
                          # bias is all-zero: h = prelu(s * dinv_d) in one op
                          nc.scalar.activation(out=h[:], in_=s[:], func=ACT.Prelu,
                                               scale=dd_t[:, g:g + 1], alpha=0.2)
                      else:
                          s2 = epp.tile([P, HID], F32, tag="s2")
                          nc.scalar.activation(out=s2[:], in_=s[:], func=ACT.Copy,
                                               scale=dd_t[:, g:g + 1])
                          s3 = epp.tile([P, HID], F32, tag="s3")
                          nc.vector.tensor_add(out=s3[:], in0=s2[:], in1=b1t[:])
                          nc.scalar.activation(out=h[:], in_=s3[:], func=ACT.Prelu,
                                               alpha=0.2)
                      pt = tpp.tile([P, P], BF16, tag="pt")
                      nc.tensor.transpose(out=pt[:], in_=h[:, 0:P], identity=ident[:])
                      hta = epp.tile([P, P], BF16, tag="hta")
                      nc.vector.tensor_copy(out=hta[:], in_=pt[:])
                      pt2 = tpp.tile([P, P], BF16, tag="pt")
                      nc.tensor.transpose(out=pt2[:], in_=h[:, P:2 * P], identity=ident[:])
                      htb = epp.tile([P, P], BF16, tag="htb")
                      nc.vector.tensor_copy(out=htb[:], in_=pt2[:])
                      ps2 = mm2p.tile([P, OUT], F32, tag="mm2")
                      nc.tensor.matmul(out=ps2[:], lhsT=hta[:], rhs=w2a[:],
                                       start=True, stop=False)
                      nc.tensor.matmul(out=ps2[:], lhsT=htb[:], rhs=w2b[:],
                                       start=False, stop=True)
                      nc.scalar.activation(out=xw2w[:, gi, :], in_=ps2[:], func=ACT.Copy,
                                           scale=dd_t[:, g:g + 1])
                  nc.scalar.dma_start(
                      out=xw2_o[g0 * P:(g0 + G_PER) * P, :]
                          .rearrange("(j p) f -> p j f", p=P),
                      in_=xw2w[:])
    nc.compile()
    return nc


def _build_b(sts, nm_tot, ic_tot, nmm, nbmax, icmax, zb2=False):
    nc = bacc.Bacc("TRN2", target_bir_lowering=False, debug=False, num_devices=8,
                   num_swdge_queues=4)
    xw2_lo = nc.dram_tensor("xw2_lo", [LO, OUT], BF16, kind="ExternalInput")
    xw2_hi = nc.dram_tensor("xw2_hi", [NHI, OUT], BF16, kind="ExternalInput")
    b2t_d = nc.dram_tensor("b2t", [P, OUT], F32, kind="ExternalInput")
    ddst_d = nc.dram_tensor("ddst", [P, NG], F32, kind="ExternalInput")
    iota_d = nc.dram_tensor("iota3", [P, nmm, P], BF16, kind="ExternalInput")
    dmod_d = nc.dram_tensor("dmod", [P, nm_tot], BF16, kind="ExternalInput")
    idx_d = nc.dram_tensor("idx", [P, ic_tot], I16, kind="ExternalInput")
    out_o = nc.dram_tensor("out_own", [HP, OUT], BF16, kind="ExternalOutput")

    ACT = mybir.ActivationFunctionType

    with tile.TileContext(nc) as tc:
        nc.gpsimd.load_library(library_config.mlp)
        qctr = [0]
        with (
            tc.tile_pool(name="const", bufs=1) as cp,
            tc.tile_pool(name="idx", bufs=3) as idxp,
            tc.tile_pool(name="data", bufs=4) as datap,
            tc.tile_pool(name="oh", bufs=4) as ohp,
            tc.tile_pool(name="agg", bufs=6, space="PSUM") as aggp,
            tc.tile_pool(name="ep", bufs=3) as epp,
        ):
            b2t = cp.tile([P, OUT], F32); nc.sync.dma_start(out=b2t[:], in_=b2t_d[:])
            dd_t = cp.tile([P, NG], F32); nc.sync.dma_start(out=dd_t[:], in_=ddst_d[:])
            iota = cp.tile([P, nmm, P], BF16); nc.sync.dma_start(out=iota[:], in_=iota_d[:])
            dmod_t = cp.tile([P, nm_tot], BF16); nc.sync.dma_start(out=dmod_t[:], in_=dmod_d[:])

            # see _build_a: clear gather buffers against 0*NaN poisoning
            for _ in range(4):
                z = datap.tile([P, nbmax, OUT], BF16, tag="data", name="zi")
                nc.vector.memset(z[:], 0.0)
            for st in sts:
                nbL, nbH = st["nbL"], st["nbH"]
                nb = nbL + nbH
                ict = st["CL"] + st["CH"]
                nmt = len(st["mms"])
                g0 = st["gs"][0]
                idx_t = idxp.tile([P, icmax], I16, tag="idx")
                nc.sync.dma_start(out=idx_t[:, 0:ict],
                                  in_=idx_d[:, st["ci0"]:st["ci0"] + ict])
                data = datap.tile([P, nbmax, OUT], BF16, tag="data")
                if st["sumSL"]:
                    _big_gather(nc, data, xw2_lo, idx_t, 0,
                                st["sumSL"], OUT, 0, qctr)
                if st["sumSH"]:
                    _big_gather(nc, data, xw2_hi, idx_t, st["CL"],
                                st["sumSH"], OUT, nbL, qctr)
                slf2 = epp.tile([P, G_PER, OUT], BF16, tag="slf")
                nc.scalar.dma_start(
                    out=slf2[:],
                    in_=xw2_lo[g0 * P:(g0 + G_PER) * P, :]
                        .rearrange("(j p) f -> p j f", p=P))
                ow = epp.tile([P, G_PER, OUT], BF16, tag="ow")
                oh = ohp.tile([P, nmm, P], FP8, tag="oh")
                h1 = max(nmt // 2, 1)
                nc.vector.tensor_tensor(
                    out=oh[:, 0:h1, :],
                    in0=dmod_t[:, st["m0"]:st["m0"] + h1].to_broadcast([P, h1, P]),
                    in1=iota[:, 0:h1, :], op=mybir.AluOpType.is_equal)
                if nmt > h1:
                    nc.vector.tensor_tensor(
                        out=oh[:, h1:nmt, :],
                        in0=dmod_t[:, st["m0"] + h1:st["m0"] + nmt]
                            .to_broadcast([P, nmt - h1, P]),
                        in1=iota[:, 0:nmt - h1, :], op=mybir.AluOpType.is_equal)
                aggs = {}
                for g in st["gs"]:
                    aggs[g] = aggp.tile([P, OUT], F32, tag="agg", name=f"agg{g}")
                for mi, (jt, g, kind, j) in enumerate(st["mms"]):
                    nc.tensor.matmul(out=aggs[g][:], lhsT=oh[:, mi, :],
                                     rhs=data[:, jt, :],
                                     start=(st["first"][g] == mi),
                                     stop=(st["last"][g] == mi))
                    if st["last"][g] != mi:
                        continue
                    gi = st["gs"].index(g)
                    s = epp.tile([P, OUT], F32, tag="s")
                    nc.vector.tensor_add(out=s[:], in0=aggs[g][:], in1=slf2[:, gi, :])
                    if zb2:
                        nc.scalar.activation(out=ow[:, gi, :], in_=s[:], func=ACT.Prelu,
                                             scale=dd_t[:, g:g + 1], alpha=0.2)
                    else:
                        s2 = epp.tile([P, OUT], F32, tag="s2")
                        nc.scalar.activation(out=s2[:], in_=s[:], func=ACT.Copy,
                                             scale=dd_t[:, g:g + 1])
                        s3 = epp.tile([P, OUT], F32, tag="s3")
                        nc.vector.tensor_add(out=s3[:], in0=s2[:], in1=b2t[:])
                        nc.scalar.activation(out=ow[:, gi, :], in_=s3[:], func=ACT.Prelu,
                                             alpha=0.2)
                nc.scalar.dma_start(
                    out=out_o[g0 * P:(g0 + G_PER) * P, :]
                        .rearrange("(j p) f -> p j f", p=P),
                    in_=ow[:])
    nc.compile()
    return nc


def kernel(**inputs):
    trace = os.environ.get("KERNEL_TRACE", "0") == "1"
    if trace:
        try:
            _install_ntff_hook()
        except Exception:
            trace = False

    f32 = np.float32
    stacks = [
        ("x_lnc_jac", "edge_jac_lnc", "W_j1_lnc", "b_j1_lnc", "W_j2_lnc", "b_j2_lnc", "W_res_lnc"),
        ("x_prot_jac", "edge_jac_prot", "W_j1_prot", "b_j1_prot", "W_j2_prot", "b_j2_prot", "W_res_prot"),
        ("x_lnc_blast", "edge_blast_lnc", "W_b1_lnc", "b_b1_lnc", "W_b2_lnc", "b_b2_lnc", None),
        ("x_prot_blast", "edge_blast_prot", "W_b1_prot", "b_b1_prot", "W_b2_prot", "b_b2_prot", None),
    ]

    # per-core edge preprocessing.  Pass 1: per-core balanced dst->row
    # permutation (equalizes per-group edge counts so the SPMD max-over-cores
    # slot padding nearly vanishes).  Pass 2: relabel edges through the
    # row maps (partner srcs use the partner core's permutation).
    raw = []
    pos_owns = []
    for c in range(8):
        sname = stacks[c // 2]
        h = c % 2
        e = np.asarray(inputs[sname[1]])
        src, dst = e[0].astype(np.int64), e[1].astype(np.int64)
        deg = (np.bincount(dst, minlength=N) + 1.0).astype(f32)
        dinv = (1.0 / np.sqrt(deg)).astype(f32)
        sel = (dst >= h * HALF) & (dst < (h + 1) * HALF)
        srcs, dsts = src[sel], dst[sel] - h * HALF
        w = np.bincount(dsts, minlength=HALF)
        pos_owns.append(_balance_pos(w))
        raw.append((srcs, dsts, dinv))

    pre = []
    cnt_lo = np.zeros((8, NG), dtype=np.int64)
    cnt_hi = np.zeros((8, NG), dtype=np.int64)
    for c in range(8):
        h = c % 2
        srcs, dsts, dinv = raw[c]
        pos_own, pos_par = pos_owns[c], pos_owns[c ^ 1]
        own = (srcs >= h * HALF) & (srcs < (h + 1) * HALF)
        src_r = np.where(own, pos_own[np.clip(srcs - h * HALF, 0, HALF - 1)],
                         HP + pos_par[np.clip(srcs - (1 - h) * HALF, 0, HALF - 1)])
        dst_l = pos_own[dsts]
        gi = dst_l // P
        ki = (src_r >= LO).astype(np.int64)
        cnt_lo[c] = np.bincount(gi[ki == 0], minlength=NG)
        cnt_hi[c] = np.bincount(gi[ki == 1], minlength=NG)
        pre.append((src_r, dst_l, dinv))

    SL, SH, sts, nm_tot, ic_tot = _build_layout(cnt_lo, cnt_hi)
    if os.environ.get("KERNEL_DEBUG", "0") == "1":
        slots = int(SL.sum() + SH.sum())
        print(f"dbg slots={slots} edges/core~{cnt_lo.sum(1).mean()+cnt_hi.sum(1).mean():.0f} "
              f"pad={slots / (cnt_lo.sum(1).mean()+cnt_hi.sum(1).mean()) - 1:.4f}",
              flush=True)
    nmm = max(len(st["mms"]) for st in sts)
    nbmax = max(st["nbL"] + st["nbH"] for st in sts)
    icmax = max(st["CL"] + st["CH"] for st in sts)

    iota3 = np.broadcast_to(np.arange(P, dtype=f32), (P, nmm, P)).astype(BF).copy()
    ident = np.eye(P, dtype=f32).astype(BF)

    in_a, in_b_partial = [], []
    for c in range(8):
        sname = stacks[c // 2]
        h = c % 2
        src_r, dst_l, dinv = pre[c]
        idx_full, dmod = _core_tables(src_r, dst_l, sts, nm_tot, ic_tot)
        x = np.asarray(inputs[sname[0]], dtype=f32)
        nodeatrow = np.full(NP, -1, dtype=np.int64)
        nodeatrow[pos_owns[c]] = h * HALF + np.arange(HALF)
        nodeatrow[HP + pos_owns[c ^ 1]] = (1 - h) * HALF + np.arange(HALF)
        valid = nodeatrow >= 0
        xT = np.zeros((F_IN, NP), dtype=f32)
        xT[:, valid] = x[nodeatrow[valid]].T
        dpad = np.ones(NP, dtype=f32)
        dpad[valid] = dinv[nodeatrow[valid]]
        dnod = dpad[np.arange(NP, dtype=np.int64).reshape(NCH, P).T].astype(f32)
        ddst = dpad[np.arange(HP, dtype=np.int64).reshape(NG, P).T].astype(f32)
        W1 = np.asarray(inputs[sname[2]], dtype=f32)
        b1 = np.asarray(inputs[sname[3]], dtype=f32)
        W2 = np.asarray(inputs[sname[4]], dtype=f32)
        b2 = np.asarray(inputs[sname[5]], dtype=f32)
        Wr = np.asarray(inputs[sname[6]], dtype=f32) if sname[6] else np.zeros((F_IN, OUT), dtype=f32)
        dmod_bf = dmod.astype(BF)
        xT_bf = xT.astype(BF)
        in_a.append({
            "xT": xT_bf, "W1": np.concatenate([W1, Wr], axis=1).astype(BF),
            "W2": W2.astype(BF),
            "b1t": np.broadcast_to(b1, (P, HID)).copy(),
            "dnod": np.ascontiguousarray(dnod), "ddst": np.ascontiguousarray(ddst),
            "iota3": iota3, "ident": ident,
            "dmod": dmod_bf, "idx": idx_full,
        })
        in_b_partial.append({
            "b2t": np.broadcast_to(b2, (P, OUT)).copy(),
            "ddst": np.ascontiguousarray(ddst),
            "iota3": iota3, "dmod": dmod_bf, "idx": idx_full,
        })

    import tempfile
    zb1 = all(not np.any(np.asarray(inputs[stacks[s][3]])) for s in range(4))
    zb2 = all(not np.any(np.asarray(inputs[stacks[s][5]])) for s in range(4))
    nc_a = _build_a(sts, nm_tot, ic_tot, nmm, nbmax, icmax, zb1=zb1)
    res_a = run_bass_kernel_spmd(nc_a, in_a, list(range(8)), trace=trace,
                                 tmpdir=tempfile.mkdtemp(prefix="gnn_a_") if trace else None)
    LAST_EXEC_NS.clear()
    if trace and res_a.exec_time_ns:
        LAST_EXEC_NS.append(res_a.exec_time_ns)

    if os.environ.get("KERNEL_DEBUG", "0") == "1":
        for c in range(8):
            xo = np.asarray(res_a.results[c]["xw2_own"]).astype(np.float32)
            ro = np.asarray(res_a.results[c]["res_own"]).astype(np.float32)
            print(f"dbg core {c}: xw2 nan%={np.isnan(xo).mean():.4f} "
                  f"max={np.nanmax(np.abs(xo)):.3f}  res nan%={np.isnan(ro).mean():.4f} "
                  f"max={np.nanmax(np.abs(ro)):.3f}", flush=True)
        xo = np.asarray(res_a.results[0]["xw2_own"]).astype(np.float32)
        nanrow = np.isnan(xo).any(axis=1)
        print("dbg nan rows core0:", np.where(nanrow)[0][:20], nanrow.sum(), flush=True)

    # host halo exchange: assemble full xw2 per pair (bf16 concat only;
    # both halves are already in their core's permuted row order)
    in_b = []
    for c in range(8):
        partner = c ^ 1
        xw2f = np.zeros((NP, OUT), dtype=BF)
        xw2f[:HP] = np.asarray(res_a.results[c]["xw2_own"])
        xw2f[HP:] = np.asarray(res_a.results[partner]["xw2_own"])
        in_b.append({"xw2_lo": xw2f[:LO].copy(), "xw2_hi": xw2f[LO:].copy(),
                     **in_b_partial[c]})

    nc_b = _build_b(sts, nm_tot, ic_tot, nmm, nbmax, icmax, zb2=zb2)
    res_b = run_bass_kernel_spmd(nc_b, in_b, list(range(8)), trace=trace,
                                 tmpdir=tempfile.mkdtemp(prefix="gnn_b_") if trace else None)
    if trace and res_b.exec_time_ns:
        LAST_EXEC_NS.append(res_b.exec_time_ns)

    def full_out(pair):
        return np.concatenate([
            np.asarray(res_b.results[2 * pair]["out_own"])[pos_owns[2 * pair]],
            np.asarray(res_b.results[2 * pair + 1]["out_own"])[pos_owns[2 * pair + 1]],
        ]).astype(f32)

    jl, jp, bl, bp = full_out(0), full_out(1), full_out(2), full_out(3)
    res_l = np.concatenate(
        [np.asarray(res_a.results[0]["res_own"])[pos_owns[0]],
         np.asarray(res_a.results[1]["res_own"])[pos_owns[1]]]).astype(f32) \
        + np.asarray(inputs["b_res_lnc"], dtype=f32)
    res_p = np.concatenate(
        [np.asarray(res_a.results[2]["res_own"])[pos_owns[2]],
         np.asarray(res_a.results[3]["res_own"])[pos_owns[3]]]).astype(f32) \
        + np.asarray(inputs["b_res_prot"], dtype=f32)
    comb_l = (jl + bl) * 0.5 + res_l
    comb_p = (jp + bp) * 0.5 + res_p
    return (comb_l, comb_p, jl, jp, bl, bp)

